# revision 30
# baseline (speedup 1.0000x reference)
"""Trainium2 Bass kernel for nn_CONVMGEmbedder (3-layer GraphConv + UnitedNorm + readout).

Strategy: dst-sharded graph partition over 8 NeuronCores.
- Node shard k = rows [k*12500, (k+1)*12500), padded to 12544 (98 blocks of 128).
- Edges live on their dst-owner core, grouped by (dst block, src bucket),
  padded to a global (SPMD-uniform) tile table.
- Table layout is half-shard interleaved: table row of node (k, i) =
  k*H + i for i < H (= 6272), and NC*H + k*H + (i-H) otherwise. The
  inter-layer AllGather is split in two (mfullA = all cores' first half-
  shards -> src buckets 0-1, mfullB -> buckets 2-3), so AG(A) overlaps the
  tail of pass B and AG(B) overlaps phase-1 gathers of the next layer.
- Pass A runs two phases per layer: phase 1 consumes buckets {0,1} into
  per-block PSUM, flushed to an SBUF partial (bf16); phase 2 consumes
  buckets {2,3} and combines.
- Aggregation: dma_gather of m[src] rows (1024-row calls - the HW limit -
  rotated across the 4 SWDGE queues), one-hot S tiles (DVE is_equal),
  PE matmuls aggT += m_e^T @ S.
- h = (aggT.T @ W) * inv_sqrt_in (ACT copy w/ per-node scale, fused row-sum).
- UnitedNorm via per-graph coefficient tile AB (one fused matmul per block)
  + per-node scalars on ACT; leaky = SLOPE*x + (1-SLOPE)*relu(x) on ACT/DVE
  with the inv_sqrt_out scale folded in (positive homogeneity).
- Readout: G^T @ h3 accumulated in PSUM, AllReduce, /cnt, leaky.
"""
import math
import os
import sys

sys.path.insert(0, "/opt/trn_rl_repo")

import numpy as np


def _cfg_real():
    return dict(
        N=100000, E=1600000, C=128, B=16, L=3, NCORES=8,
        NBUCK=4, CH=8, IB=8, GD="bf16",
    )


def _derive(cfg):
    c = dict(cfg)
    c["SHARD"] = c["N"] // c["NCORES"]
    c["NBLK"] = (c["SHARD"] + 127) // 128
    c["SHARD_PAD"] = c["NBLK"] * 128
    c["NROWS"] = c["NCORES"] * c["SHARD_PAD"]
    assert c["NBLK"] % 2 == 0
    c["H"] = c["SHARD_PAD"] // 2          # half-shard rows
    c["HBLK"] = c["NBLK"] // 2
    assert c["NROWS"] % c["NBUCK"] == 0
    c["WIN"] = c["NROWS"] // c["NBUCK"]
    assert c["WIN"] <= 32768, c["WIN"]
    assert c["NCORES"] * c["H"] == 2 * c["WIN"]
    c["EPS"] = 1e-5
    c["SLOPE"] = 0.2
    return c


def prep_host(inputs, cfg):
    """Pure-numpy sharding prep: degrees, edge reorder, tile tables, constants."""
    N, E, C, B = cfg["N"], cfg["E"], cfg["C"], cfg["B"]
    NC, NBUCK = cfg["NCORES"], cfg["NBUCK"]
    SHARD, NBLK, H = cfg["SHARD"], cfg["NBLK"], cfg["H"]
    SHARD_PAD, WIN = cfg["SHARD_PAD"], cfg["WIN"]

    nf = np.asarray(inputs["node_feats"], np.float32)
    W = np.asarray(inputs["W"], np.float32)
    gamma = np.asarray(inputs["gamma"], np.float32)
    beta = np.asarray(inputs["beta"], np.float32)
    lam = np.asarray(inputs["lambdas"], np.float32)
    src = np.asarray(inputs["src"]).astype(np.int64)
    dst = np.asarray(inputs["dst"]).astype(np.int64)
    gid = np.asarray(inputs["graph_ids"]).astype(np.int64)

    deg_out = np.maximum(np.bincount(src, minlength=N).astype(np.float64), 1.0)
    deg_in = np.maximum(np.bincount(dst, minlength=N).astype(np.float64), 1.0)
    iso = (1.0 / np.sqrt(deg_out)).astype(np.float32)   # inv_sqrt_out per node
    isi = (1.0 / np.sqrt(deg_in)).astype(np.float32)    # inv_sqrt_in per node
    cnt = np.maximum(np.bincount(gid, minlength=B).astype(np.float64), 1.0)
    cnt_inv = (1.0 / cnt).astype(np.float32).reshape(B, 1)

    # layer-0 gather table: m1 = nf * iso in the half-shard interleaved layout
    m1 = nf * iso[:, None]
    m1tab = np.zeros((cfg["NROWS"], C), np.float32)
    for k in range(NC):
        sh = m1[k * SHARD:(k + 1) * SHARD]          # [12500, C]
        m1tab[k * H:k * H + H] = sh[:H]
        m1tab[NC * H + k * H:NC * H + k * H + (SHARD - H)] = sh[H:]

    # softmax(lambdas) per layer, host-side (3x3 input params)
    lam64 = lam.astype(np.float64)
    ex = np.exp(lam64 - lam64.max(axis=1, keepdims=True))
    wsoft = (ex / ex.sum(axis=1, keepdims=True)).astype(np.float64)  # [L,3]

    # edge -> (core, block, slot, bucket, idx16); table row in interleaved layout
    core = dst // SHARD
    local = dst - core * SHARD
    blk = local // 128
    slot = (local % 128).astype(np.float32)
    sk = src // SHARD
    si = src - sk * SHARD
    row = np.where(si < H, sk * H + si, NC * H + sk * H + (si - H))
    buck = row // WIN
    idx16 = (row % WIN).astype(np.int16)

    # counts per (core, blk, buck)
    key = (core * NBLK + blk) * NBUCK + buck
    cnts = np.bincount(key, minlength=NC * NBLK * NBUCK).reshape(NC, NBLK, NBUCK)
    T = np.ceil(cnts.max(axis=0) / 128.0).astype(np.int64)  # [NBLK, NBUCK]
    # each phase of each block needs >=1 tile so PSUM gets a start matmul
    T[(T[:, 0] + T[:, 1]) == 0, 0] = 1
    T[(T[:, 2] + T[:, 3]) == 0, 2] = 1

    TQ = T.sum(axis=0)          # tiles per bucket stream
    EQ = TQ * 128               # padded edges per stream
    # slot offset of (blk) within stream q: running sum of T[:, q]
    off_blk = np.zeros((NBLK, NBUCK), np.int64)
    off_blk[1:] = np.cumsum(T[:-1] * 128, axis=0)

    order = np.lexsort((src, buck, blk, core))  # (core, blk, buck), src-sorted
    per_core = []
    for k in range(NC):
        sel = order[core[order] == k]
        bblk, bbuck = blk[sel], buck[sel]
        # position within (blk, buck) group
        grp = bblk * NBUCK + bbuck
        # stable order -> rank within group
        rank = np.zeros(len(sel), np.int64)
        if len(sel):
            gcnt = np.bincount(grp, minlength=NBLK * NBUCK)
            starts = np.concatenate([[0], np.cumsum(gcnt)[:-1]])
            # edges are sorted by grp already (lexsort by (blk,buck))
            rank = np.arange(len(sel)) - starts[grp]
        pos = off_blk[bblk, bbuck] + rank           # slot within stream bbuck
        d = {}
        for q in range(NBUCK):
            eq = int(EQ[q])
            idx_q = np.zeros(eq, np.int16)
            slot_q = -np.ones(eq, np.float32)
            m = bbuck == q
            idx_q[pos[m]] = idx16[sel[m]]
            slot_q[pos[m]] = slot[sel[m]]
            d[f"idxq{q}"] = np.tile(
                np.ascontiguousarray(idx_q.reshape(-1, 16).T), (8, 1))
            d[f"slotq{q}"] = np.ascontiguousarray(slot_q.reshape(-1, 128).T)
        # per-node columns for this shard (padded rows -> 1.0 / gid 0)
        lo, hi = k * SHARD, (k + 1) * SHARD
        pad = SHARD_PAD - SHARD
        isi_k = np.concatenate([isi[lo:hi], np.ones(pad, np.float32)])
        iso_k = np.concatenate([iso[lo:hi], np.ones(pad, np.float32)])
        d["inv_in_c"] = np.ascontiguousarray(isi_k.reshape(NBLK, 128).T)
        d["inv_out_c"] = np.ascontiguousarray(iso_k.reshape(NBLK, 128).T)
        gid_k = gid[lo:hi]
        G = np.zeros((SHARD_PAD, B), np.float32)
        G[np.arange(SHARD), gid_k] = 1.0
        G3 = G.reshape(NBLK, 128, B)
        d["g_oh"] = np.ascontiguousarray(G3.transpose(1, 0, 2)).reshape(128, NBLK * B)
        d["g_ohT"] = np.ascontiguousarray(G3.transpose(2, 0, 1)).reshape(B, NBLK * 128)
        per_core.append(d)

    consts = dict(
        iota=np.broadcast_to(np.arange(128, dtype=np.float32), (128, 128)).copy(),
        wmat=np.ascontiguousarray(W.transpose(1, 0, 2)).reshape(C, cfg["L"] * C),
        cnt_inv=cnt_inv,
        m1tab=m1tab,
        gamma=gamma, beta=beta,
    )
    gamma_trivial = bool(np.all(gamma == 1.0) and np.all(beta == 0.0))
    meta = dict(T=T, TQ=TQ, EQ=EQ, wsoft=wsoft, gamma_trivial=gamma_trivial,
                TMAX=int(T.max()))
    return meta, per_core, consts


def build_nc(cfg, meta):
    import concourse.bacc as bacc
    import concourse.bass as bass
    import concourse.mybir as mybir
    import concourse.tile as tile

    f32 = mybir.dt.float32
    GD = f32 if cfg["GD"] == "f32" else mybir.dt.bfloat16
    C, B, L = cfg["C"], cfg["B"], cfg["L"]
    NC, NBUCK, CH, IB = cfg["NCORES"], cfg["NBUCK"], cfg["CH"], cfg["IB"]
    NBLK, SHARD_PAD, NROWS, WIN = (cfg["NBLK"], cfg["SHARD_PAD"],
                                   cfg["NROWS"], cfg["WIN"])
    H, HBLK = cfg["H"], cfg["HBLK"]
    EPS, SLOPE, N = cfg["EPS"], cfg["SLOPE"], cfg["N"]
    T, TQ, EQ = meta["T"], meta["TQ"], meta["EQ"]
    wsoft, TMAX = meta["wsoft"], meta["TMAX"]
    gtriv = meta["gamma_trivial"]
    RG = [list(range(NC))]
    eq_ = mybir.AluOpType
    AF = mybir.ActivationFunctionType

    nc = bacc.Bacc("TRN2", target_bir_lowering=False, debug=False,
                   num_devices=NC, num_swdge_queues=4)

    # ---- DRAM tensors ----
    m1tab_t = nc.dram_tensor("m1tab", [NROWS, C], GD, kind="ExternalInput")
    out_t = nc.dram_tensor("out", [B, C], f32, kind="ExternalOutput")
    idx_t, slot_t = [], []
    for q in range(NBUCK):
        idx_t.append(nc.dram_tensor(f"idxq{q}", [128, int(EQ[q]) // 16],
                                    mybir.dt.int16, kind="ExternalInput"))
        slot_t.append(nc.dram_tensor(f"slotq{q}", [128, int(EQ[q]) // 128],
                                     GD, kind="ExternalInput"))
    invin_t = nc.dram_tensor("inv_in_c", [128, NBLK], f32, kind="ExternalInput")
    invout_t = nc.dram_tensor("inv_out_c", [128, NBLK], f32, kind="ExternalInput")
    goh_t = nc.dram_tensor("g_oh", [128, NBLK * B], f32, kind="ExternalInput")
    gohT_t = nc.dram_tensor("g_ohT", [B, NBLK * 128], f32, kind="ExternalInput")
    iota_t = nc.dram_tensor("iota", [128, 128], GD, kind="ExternalInput")
    wmat_t = nc.dram_tensor("wmat", [C, L * C], f32, kind="ExternalInput")
    cntinv_t = nc.dram_tensor("cnt_inv", [B, 1], f32, kind="ExternalInput")
    gamma_t = nc.dram_tensor("gamma", [L, C], f32, kind="ExternalInput")
    beta_t = nc.dram_tensor("beta", [L, C], f32, kind="ExternalInput")

    # per-layer tables: layer 0 reads m1tab; layers 1.. read the AG outputs
    mshA, mshB, mfA, mfB, stin, stout = [], [], [], [], [], []
    for l in range(L - 1):
        mshA.append(nc.dram_tensor(f"mshA{l}", [H, C], GD))
        mshB.append(nc.dram_tensor(f"mshB{l}", [H, C], GD))
        mfA.append(nc.dram_tensor(f"mfA{l}", [NC * H, C], GD,
                                  addr_space="Shared"))
        mfB.append(nc.dram_tensor(f"mfB{l}", [NC * H, C], GD,
                                  addr_space="Shared"))
    for l in range(L):
        stin.append(nc.dram_tensor(f"stin{l}", [B, 2 * C], f32))
        stout.append(nc.dram_tensor(f"stout{l}", [B, 2 * C], f32,
                                    addr_space="Shared"))
    embin = nc.dram_tensor("embin", [B, C], f32)
    embout = nc.dram_tensor("embout", [B, C], f32, addr_space="Shared")

    def tab_ap(l, q):
        if l == 0:
            return m1tab_t.ap()[q * WIN:(q + 1) * WIN, :]
        if q < 2:
            return mfA[l - 1].ap()[q * WIN:(q + 1) * WIN, :]
        return mfB[l - 1].ap()[(q - 2) * WIN:(q - 1) * WIN, :]

    with tile.TileContext(nc) as tc:
        with (
            tc.tile_pool(name="const", bufs=1) as cp,
            tc.tile_pool(name="big", bufs=1) as bigp,
            tc.tile_pool(name="gath", bufs=2) as gp,
            tc.tile_pool(name="work", bufs=3) as wp,
            tc.tile_pool(name="coef", bufs=1) as kp,
            tc.tile_pool(name="psum", bufs=2, space="PSUM") as pp,
            tc.tile_pool(name="psum1", bufs=1, space="PSUM") as pp1,
        ):
            # ---- resident constants ----
            iota = cp.tile([128, 128], GD)
            nc.sync.dma_start(iota[:], iota_t.ap())
            wm = cp.tile([C, L, C], f32)
            nc.sync.dma_start(wm[:], wmat_t.ap().rearrange("c (l k) -> c l k", l=L))
            goh = cp.tile([128, NBLK, B], f32)
            nc.sync.dma_start(goh[:], goh_t.ap().rearrange("p (b g) -> p b g", b=NBLK))
            goh16 = cp.tile([128, NBLK, B], GD)
            nc.vector.tensor_copy(goh16[:], goh[:])
            gTall = cp.tile([B, NBLK, 128], f32)
            nc.sync.dma_start(gTall[:],
                              gohT_t.ap().rearrange("g (b p) -> g b p", b=NBLK))
            invin = cp.tile([128, NBLK], f32)
            nc.sync.dma_start(invin[:], invin_t.ap())
            invout = cp.tile([128, NBLK], f32)
            nc.sync.dma_start(invout[:], invout_t.ap())
            invout02 = cp.tile([128, NBLK], f32)
            nc.vector.tensor_scalar_mul(invout02[:], invout[:], SLOPE)
            invout08 = cp.tile([128, NBLK], f32)
            nc.vector.tensor_scalar_mul(invout08[:], invout[:], 1.0 - SLOPE)
            cntinv = cp.tile([B, 1], f32)
            nc.sync.dma_start(cntinv[:], cntinv_t.ap())
            ones16 = cp.tile([B, 1], f32)
            nc.vector.memset(ones16[:], 1.0)
            ones1 = cp.tile([1, B], f32)
            nc.vector.memset(ones1[:], 1.0)
            ones1p = cp.tile([1, 128], f32)
            nc.vector.memset(ones1p[:], 1.0)
            eps128 = cp.tile([128, 1], f32)
            nc.vector.memset(eps128[:], EPS)
            gam = cp.tile([L, C], f32)
            nc.sync.dma_start(gam[:], gamma_t.ap())
            bet = cp.tile([L, C], f32)
            nc.sync.dma_start(bet[:], beta_t.ap())

            qctr = [0]  # unused placeholder (queues rewritten post-compile)
            # hbuf: h per block in pass A; overwritten in-place by pass B
            # with the next layer's m rows (h is dead once consumed).
            hbuf = bigp.tile([128, NBLK, C], GD)
            aggP = bigp.tile([C, NBLK, 128], GD)     # phase-1 partial aggT
            nm_arr = cp.tile([128, NBLK], f32)
            nv_arr = cp.tile([128, NBLK], f32)


            for l in range(L):
                w0, w1, w2 = [float(x) for x in wsoft[l]]
                gs_p = pp1.tile([B, C], f32, tag="gs")
                gss_p = pp1.tile([B, C], f32, tag="gss")
                cur = [0] * NBUCK          # consumed tiles per stream
                chunk_tiles = [None] * NBUCK
                chunk_id = [-1] * NBUCK
                ibatch_tiles = [None] * NBUCK
                ibatch_id = [-1] * NBUCK

                def issue_chunk(q, ci, l=l, ibatch_tiles=ibatch_tiles,
                                ibatch_id=ibatch_id):
                    bi = ci // IB
                    if ibatch_id[q] != bi:
                        nt_b = min(IB * CH, int(TQ[q]) - bi * IB * CH)
                        it = gp.tile([128, IB * CH * 8], mybir.dt.int16,
                                     tag=f"i{q}", name=f"it{q}")
                        c0 = bi * IB * CH
                        nc.sync.dma_start(it[:, :nt_b * 8],
                                          idx_t[q].ap()[:, c0 * 8:(c0 + nt_b) * 8])
                        st = gp.tile([128, IB * CH], GD, tag=f"s{q}", name=f"st{q}")
                        nc.sync.dma_start(st[:, :nt_b],
                                          slot_t[q].ap()[:, c0:c0 + nt_b])
                        ibatch_tiles[q] = (it, st)
                        ibatch_id[q] = bi
                    it, st = ibatch_tiles[q]
                    r = min(CH, int(TQ[q]) - ci * CH)
                    co = (ci % IB) * CH
                    gt = gp.tile([128, CH, C], GD, tag=f"g{q}")
                    nc.gpsimd.dma_gather(
                        gt[:, :r, :], tab_ap(l, q),
                        it[:, co * 8:(co + r) * 8],
                        r * 128, r * 128, C, queue_num=0,
                        single_packet=False)
                    S = gp.tile([128, CH, 128], GD, tag=f"S{q}", name=f"S{q}")
                    nc.vector.tensor_tensor(
                        out=S[:, :r, :],
                        in0=iota[:].unsqueeze(1).broadcast_to([128, r, 128]),
                        in1=st[:, co:co + r].unsqueeze(2)
                            .broadcast_to([128, r, 128]),
                        op=eq_.is_equal)
                    return (gt, S)

                def consume(b, qs, aggT_p):
                    ntot = int(sum(T[b, q] for q in qs))
                    done = 0
                    for q in qs:
                        nt = int(T[b, q])
                        t0 = cur[q]
                        cur[q] += nt
                        while nt > 0:
                            ci = t0 // CH
                            if chunk_id[q] != ci:
                                chunk_tiles[q] = issue_chunk(q, ci)
                                chunk_id[q] = ci
                            col = t0 % CH
                            r = min(nt, CH - col)
                            gt, S = chunk_tiles[q]
                            for j in range(r):
                                nc.tensor.matmul(
                                    aggT_p[:], gt[:, col + j, :], S[:, col + j, :],
                                    start=(done == 0), stop=(done == ntot - 1))
                                done += 1
                            t0 += r
                            nt -= r

                # ---------------- PASS A phase 1: buckets 0,1 ----------------
                scope1 = nc.named_scope(f"passA1_{l}"); scope1.__enter__()
                for b in range(NBLK):
                    aggT_p = pp.tile([C, 128], f32, tag="aggT")
                    consume(b, (0, 1), aggT_p)
                    nc.scalar.activation(aggP[:, b, :], aggT_p[:], AF.Copy)
                scope1.__exit__(None, None, None)

                # ---------------- PASS A phase 2: buckets 2,3 ----------------
                scope2 = nc.named_scope(f"passA2_{l}"); scope2.__enter__()
                for b in range(NBLK):
                    aggT_p = pp.tile([C, 128], f32, tag="aggT")
                    consume(b, (2, 3), aggT_p)
                    aggT_s = wp.tile([C, 128], f32, tag="aggTs")
                    nc.vector.tensor_tensor(out=aggT_s[:], in0=aggT_p[:],
                                            in1=aggP[:, b, :], op=eq_.add)
                    h_p = pp.tile([128, C], f32, tag="hp")
                    nc.tensor.matmul(h_p[:], aggT_s[:], wm[:, l, :],
                                     start=True, stop=True)
                    nc.scalar.activation(hbuf[:, b, :], h_p[:], AF.Copy,
                                         scale=invin[:, b:b + 1],
                                         accum_out=nm_arr[:, b:b + 1])
                    h2 = wp.tile([128, C], GD, tag="h2")
                    nc.scalar.activation(h2[:], hbuf[:, b, :], AF.Square,
                                         accum_out=nv_arr[:, b:b + 1])
                    nc.tensor.matmul(gs_p[:], goh16[:, b, :], hbuf[:, b, :],
                                     start=(b == 0), stop=(b == NBLK - 1))
                    nc.tensor.matmul(gss_p[:], goh16[:, b, :], h2[:],
                                     start=(b == 0), stop=(b == NBLK - 1))
                scope2.__exit__(None, None, None)

                scopeS = nc.named_scope(f"stats_{l}"); scopeS.__enter__()
                # ---- stats AllReduce ----
                sts = kp.tile([B, 2 * C], f32, tag="sts")
                nc.vector.tensor_copy(sts[:, 0:C], gs_p[:])
                nc.vector.tensor_copy(sts[:, C:2 * C], gss_p[:])
                nc.sync.dma_start(stin[l].ap(), sts[:])
                nc.gpsimd.collective_compute(
                    "AllReduce", eq_.add, ins=[stin[l].ap()],
                    outs=[stout[l].ap()], replica_groups=RG)
                gst = kp.tile([B, 2 * C], f32, tag="gst")
                nc.sync.dma_start(gst[:], stout[l].ap())
                gs, gss = gst[:, 0:C], gst[:, C:2 * C]

                # ---- coefficients AB = [A16 | B16] (bf16) ----
                gm = kp.tile([B, C], f32, tag="gm")
                nc.vector.tensor_scalar_mul(gm[:], gs, cntinv[:])
                gv = kp.tile([B, C], f32, tag="gv")
                nc.vector.tensor_scalar_mul(gv[:], gss, cntinv[:])
                tmp16 = kp.tile([B, C], f32, tag="tmp16")
                nc.vector.tensor_tensor(out=tmp16[:], in0=gm[:], in1=gm[:], op=eq_.mult)
                nc.vector.tensor_tensor(out=gv[:], in0=gv[:], in1=tmp16[:], op=eq_.subtract)
                nc.scalar.activation(gv[:], gv[:], AF.Sqrt, bias=eps128[0:B, :])
                igv = kp.tile([B, C], f32, tag="igv")
                nc.vector.reciprocal(igv[:], gv[:])
                bs_p = pp.tile([1, 2 * C], f32, tag="aggT")
                nc.tensor.matmul(bs_p[:], ones16[:], gst[:], start=True, stop=True)
                bm = kp.tile([1, C], f32, tag="bm")
                nc.vector.tensor_scalar_mul(bm[:], bs_p[:, 0:C], 1.0 / N)
                bv = kp.tile([1, C], f32, tag="bv")
                nc.vector.tensor_scalar_mul(bv[:], bs_p[:, C:2 * C], 1.0 / N)
                tmp1 = kp.tile([1, C], f32, tag="tmp1")
                nc.vector.tensor_tensor(out=tmp1[:], in0=bm[:], in1=bm[:], op=eq_.mult)
                nc.vector.tensor_tensor(out=bv[:], in0=bv[:], in1=tmp1[:], op=eq_.subtract)
                nc.scalar.activation(bv[:], bv[:], AF.Sqrt, bias=eps128[0:1, :])
                ibv = kp.tile([1, C], f32, tag="ibv")
                nc.vector.reciprocal(ibv[:], bv[:])
                # broadcast [1,C] rows to B partitions via K=1 matmul
                ibv_p = pp.tile([B, C], f32, tag="aggT")
                nc.tensor.matmul(ibv_p[:], ones1[:], ibv[:], start=True, stop=True)
                bmibv = kp.tile([1, C], f32, tag="bmibv")
                nc.vector.tensor_tensor(out=bmibv[:], in0=bm[:], in1=ibv[:], op=eq_.mult)
                bmibv_p = pp.tile([B, C], f32, tag="hp")
                nc.tensor.matmul(bmibv_p[:], ones1[:], bmibv[:], start=True, stop=True)
                AB = kp.tile([B, 2 * C], f32, tag="AB")
                A16, B16 = AB[:, 0:C], AB[:, C:2 * C]
                nc.vector.tensor_scalar_mul(A16, igv[:], w1)
                t16b = kp.tile([B, C], f32, tag="t16b")
                nc.vector.tensor_scalar_mul(t16b[:], ibv_p[:], w0)
                nc.vector.tensor_tensor(out=A16, in0=A16, in1=t16b[:], op=eq_.add)
                nc.vector.tensor_tensor(out=B16, in0=gm[:], in1=igv[:], op=eq_.mult)
                nc.vector.tensor_scalar_mul(B16, B16, w1)
                nc.vector.tensor_scalar_mul(t16b[:], bmibv_p[:], w0)
                nc.vector.tensor_tensor(out=B16, in0=B16, in1=t16b[:], op=eq_.add)

                # ---- per-node coefficients: a_n = w2*invn, bn_neg = -nm*a_n ----
                nmm = kp.tile([128, NBLK], f32, tag="nmm")
                nc.vector.tensor_scalar_mul(nmm[:], nm_arr[:], 1.0 / C)
                nvm = kp.tile([128, NBLK], f32, tag="nvm")
                nc.vector.tensor_scalar_mul(nvm[:], nv_arr[:], 1.0 / C)
                nm2 = kp.tile([128, NBLK], f32, tag="nm2")
                nc.vector.tensor_tensor(out=nm2[:], in0=nmm[:], in1=nmm[:], op=eq_.mult)
                nc.vector.tensor_tensor(out=nvm[:], in0=nvm[:], in1=nm2[:], op=eq_.subtract)
                nc.scalar.activation(nvm[:], nvm[:], AF.Sqrt, bias=eps128[:])
                invn = kp.tile([128, NBLK], f32, tag="invn")
                nc.vector.reciprocal(invn[:], nvm[:])
                a_n = kp.tile([128, NBLK], f32, tag="a_n")
                nc.vector.tensor_scalar_mul(a_n[:], invn[:], w2)
                bn_neg = kp.tile([128, NBLK], f32, tag="bn_neg")
                nc.vector.tensor_tensor(out=bn_neg[:], in0=nmm[:], in1=a_n[:], op=eq_.mult)
                nc.vector.tensor_scalar_mul(bn_neg[:], bn_neg[:], -1.0)

                scopeS.__exit__(None, None, None)
                if l == L - 1:
                    emb_p = pp1.tile([B, C], f32, tag="emb")

                # ---------------- PASS B ----------------
                scopeB = nc.named_scope(f"passB_{l}"); scopeB.__enter__()
                for b in range(NBLK):
                    AB_p = pp.tile([128, 2 * C], f32, tag="aggT")
                    nc.tensor.matmul(AB_p[:], gTall[:, b, :], AB[:],
                                     start=True, stop=True)
                    h = hbuf[:, b, :]
                    # w = a_n*h - b_n   (ACT, per-node scale/bias)
                    w_t = wp.tile([128, C], f32, tag="w_t")
                    nc.scalar.activation(w_t[:], h, AF.Identity,
                                         scale=a_n[:, b:b + 1],
                                         bias=bn_neg[:, b:b + 1])
                    # v = h*A_p - B_p  (DVE, PSUM operands)
                    v = wp.tile([128, C], f32, tag="v")
                    nc.vector.tensor_tensor(out=v[:], in0=h, in1=AB_p[:, 0:C], op=eq_.mult)
                    nc.vector.tensor_tensor(out=v[:], in0=v[:], in1=AB_p[:, C:2 * C],
                                            op=eq_.subtract)
                    u = wp.tile([128, C], f32, tag="u")
                    nc.vector.tensor_tensor(out=u[:], in0=v[:], in1=w_t[:], op=eq_.add)
                    if not gtriv:
                        gam_p = pp.tile([128, C], f32, tag="hp")
                        nc.tensor.matmul(gam_p[:], ones1p[:], gam[l:l + 1, :],
                                         start=True, stop=True)
                        bet_p = pp.tile([128, C], f32, tag="hp2")
                        nc.tensor.matmul(bet_p[:], ones1p[:], bet[l:l + 1, :],
                                         start=True, stop=True)
                        nc.vector.tensor_tensor(out=u[:], in0=u[:], in1=gam_p[:], op=eq_.mult)
                        nc.vector.tensor_tensor(out=u[:], in0=u[:], in1=bet_p[:], op=eq_.add)
                    if l < L - 1:
                        # m = leaky(u)*invout = SLOPE*invout*u + (1-SLOPE)*relu(u*invout)
                        r_t = wp.tile([128, C], f32, tag="r_t")
                        nc.scalar.activation(r_t[:], u[:], AF.Relu,
                                             scale=invout08[:, b:b + 1])
                        nc.vector.tensor_scalar(u[:], u[:],
                                                invout02[:, b:b + 1], None,
                                                eq_.mult)
                        nc.vector.tensor_tensor(out=hbuf[:, b, :], in0=u[:],
                                                in1=r_t[:], op=eq_.add)
                        if b == HBLK - 1:
                            nc.sync.dma_start(
                                mshA[l].ap().rearrange("(b p) c -> p b c", p=128),
                                hbuf[:, 0:HBLK, :])
                            nc.gpsimd.collective_compute(
                                "AllGather", eq_.bypass, ins=[mshA[l].ap()],
                                outs=[mfA[l].ap()], replica_groups=RG)
                        elif b == NBLK - 1:
                            nc.sync.dma_start(
                                mshB[l].ap().rearrange("(b p) c -> p b c", p=128),
                                hbuf[:, HBLK:NBLK, :])
                            nc.gpsimd.collective_compute(
                                "AllGather", eq_.bypass, ins=[mshB[l].ap()],
                                outs=[mfB[l].ap()], replica_groups=RG)
                    else:
                        r_t = wp.tile([128, C], f32, tag="r_t")
                        nc.scalar.activation(r_t[:], u[:], AF.Relu,
                                             scale=1.0 - SLOPE)
                        nc.vector.tensor_scalar_mul(u[:], u[:], SLOPE)
                        lu = wp.tile([128, C], f32, tag="lu")
                        nc.vector.tensor_tensor(out=lu[:], in0=u[:],
                                                in1=r_t[:], op=eq_.add)
                        nc.tensor.matmul(emb_p[:], goh[:, b, :], lu[:],
                                         start=(b == 0), stop=(b == NBLK - 1))
                scopeB.__exit__(None, None, None)

            # ---- readout ----
            embs = kp.tile([B, C], f32, tag="embs")
            nc.vector.tensor_copy(embs[:], emb_p[:])
            nc.sync.dma_start(embin.ap(), embs[:])
            nc.gpsimd.collective_compute(
                "AllReduce", eq_.add, ins=[embin.ap()], outs=[embout.ap()],
                replica_groups=RG)
            embg = kp.tile([B, C], f32, tag="embg")
            nc.sync.dma_start(embg[:], embout.ap())
            nc.vector.tensor_scalar_mul(embg[:], embg[:], cntinv[:])
            embg2 = kp.tile([B, C], f32, tag="embg2")
            nc.vector.tensor_scalar_mul(embg2[:], embg[:], SLOPE)
            nc.vector.tensor_tensor(out=embg[:], in0=embg[:], in1=embg2[:], op=eq_.max)
            nc.sync.dma_start(out_t.ap(), embg[:])

    nc.compile()
    _fixup_swdge_queues(nc)
    return nc


def _fixup_swdge_queues(nc):
    """Rewrite gather queue_num in scheduled (module) order.

    Tile assigns DMASW completion-sem lanes round-robin (mod 8) over
    Pool-engine DMA instructions in module order, and the ucode requires a
    sem lane to be incremented from a single SWDGE queue. Setting
    queue = lane % 4 in the same walk order satisfies that for any schedule
    while keeping all 4 queues busy.
    """
    import concourse.mybir as mybir
    pool_dma = ("InstDMAGatherAnt", "InstDMAScatterAddAnt",
                "InstKVWritebackAnt", "InstPagedWritebackAnt")
    lane = 0
    for f in nc.m.functions:
        for bb in f.blocks:
            for inst in bb.instructions:
                cls = type(inst).__name__
                is_pool_dma = cls in pool_dma or (
                    cls in ("InstDMACopy", "InstTensorLoad", "InstTensorSave")
                    and getattr(inst, "engine", None) == mybir.EngineType.Pool)
                if is_pool_dma:
                    inst.queue_num = lane % 4
                    lane += 1


def make_in_maps(cfg, per_core, consts):
    import ml_dtypes
    GD_np = np.float32 if cfg["GD"] == "f32" else ml_dtypes.bfloat16
    base = dict(
        m1tab=consts["m1tab"].astype(GD_np),
        iota=consts["iota"].astype(GD_np), wmat=consts["wmat"],
        cnt_inv=consts["cnt_inv"], gamma=consts["gamma"], beta=consts["beta"],
    )
    in_maps = []
    for k in range(cfg["NCORES"]):
        d = dict(base)
        for kk, vv in per_core[k].items():
            if kk.startswith("slotq"):
                vv = vv.astype(GD_np)
            d[kk] = vv
        in_maps.append(d)
    return in_maps


_BUILD_CACHE = {}


def _build_cached(cfg, meta):
    key = (tuple(sorted((k, str(v)) for k, v in cfg.items())),
           meta["T"].tobytes(), meta["wsoft"].tobytes(), meta["gamma_trivial"])
    if key not in _BUILD_CACHE:
        _BUILD_CACHE[key] = build_nc(cfg, meta)
    return _BUILD_CACHE[key]


def run_cfg(cfg, inputs, trace=False, verbose=False):
    import time
    from concourse.bass_utils import run_bass_kernel_spmd
    t0 = time.time()
    cfg = _derive(cfg)
    meta, per_core, consts = prep_host(inputs, cfg)
    t1 = time.time()
    nc = _build_cached(cfg, meta)
    t2 = time.time()
    in_maps = make_in_maps(cfg, per_core, consts)
    res = run_bass_kernel_spmd(nc, in_maps, list(range(cfg["NCORES"])),
                               trace=trace)
    t3 = time.time()
    if verbose:
        print(f"[timing] prep={t1-t0:.1f}s build+compile={t2-t1:.1f}s "
              f"run={t3-t2:.1f}s", flush=True)
    return res.results[0]["out"].astype(np.float32), res


def kernel(**inputs):
    out, _ = run_cfg(_cfg_real(), inputs)
    return out


# revision 33
# speedup vs baseline: 1.3805x; 1.3805x over previous
"""Trainium2 Bass kernel for nn_CONVMGEmbedder (3-layer GraphConv + UnitedNorm + readout).

Strategy: dst-sharded graph partition over 8 NeuronCores.
- Node shard k = rows [k*12500, (k+1)*12500), padded to 12544 (98 blocks of 128).
- Edges live on their dst-owner core, grouped by (dst block, src bucket),
  padded to a global (SPMD-uniform) tile table.
- Table layout is half-shard interleaved: table row of node (k, i) =
  k*H + i for i < H (= 6272), and NC*H + k*H + (i-H) otherwise. The
  inter-layer AllGather is split in two (mfullA = all cores' first half-
  shards -> src buckets 0-1, mfullB -> buckets 2-3), so AG(A) overlaps the
  tail of pass B and AG(B) overlaps phase-1 gathers of the next layer.
- Pass A runs two phases per layer: phase 1 consumes buckets {0,1} into
  per-block PSUM, flushed to an SBUF partial (bf16); phase 2 consumes
  buckets {2,3} and combines.
- Aggregation: dma_gather of m[src] rows (1024-row calls - the HW limit -
  rotated across the 4 SWDGE queues), one-hot S tiles (DVE is_equal),
  PE matmuls aggT += m_e^T @ S.
- h = (aggT.T @ W) * inv_sqrt_in (ACT copy w/ per-node scale, fused row-sum).
- UnitedNorm via per-graph coefficient tile AB (one fused matmul per block)
  + per-node scalars on ACT; leaky = SLOPE*x + (1-SLOPE)*relu(x) on ACT/DVE
  with the inv_sqrt_out scale folded in (positive homogeneity).
- Readout: G^T @ h3 accumulated in PSUM, AllReduce, /cnt, leaky.
"""
import math
import os
import sys

sys.path.insert(0, "/opt/trn_rl_repo")

import numpy as np


def _cfg_real():
    return dict(
        N=100000, E=1600000, C=128, B=16, L=3, NCORES=8,
        NBUCK=4, CH=8, IB=8, GD="bf16",
    )


def _derive(cfg):
    c = dict(cfg)
    c["SHARD"] = c["N"] // c["NCORES"]
    c["NBLK"] = (c["SHARD"] + 127) // 128
    c["SHARD_PAD"] = c["NBLK"] * 128
    c["NROWS"] = c["NCORES"] * c["SHARD_PAD"]
    assert c["NBLK"] % 2 == 0
    c["H"] = c["SHARD_PAD"] // 2          # half-shard rows
    c["HBLK"] = c["NBLK"] // 2
    assert c["NROWS"] % c["NBUCK"] == 0
    c["WIN"] = c["NROWS"] // c["NBUCK"]
    assert c["WIN"] <= 32768, c["WIN"]
    assert c["NCORES"] * c["H"] == 2 * c["WIN"]
    c["EPS"] = 1e-5
    c["SLOPE"] = 0.2
    return c


def prep_host(inputs, cfg):
    """Pure-numpy sharding prep: degrees, edge reorder, tile tables, constants."""
    N, E, C, B = cfg["N"], cfg["E"], cfg["C"], cfg["B"]
    NC, NBUCK = cfg["NCORES"], cfg["NBUCK"]
    SHARD, NBLK, H = cfg["SHARD"], cfg["NBLK"], cfg["H"]
    SHARD_PAD, WIN = cfg["SHARD_PAD"], cfg["WIN"]

    nf = np.asarray(inputs["node_feats"], np.float32)
    W = np.asarray(inputs["W"], np.float32)
    gamma = np.asarray(inputs["gamma"], np.float32)
    beta = np.asarray(inputs["beta"], np.float32)
    lam = np.asarray(inputs["lambdas"], np.float32)
    src = np.asarray(inputs["src"]).astype(np.int64)
    dst = np.asarray(inputs["dst"]).astype(np.int64)
    gid = np.asarray(inputs["graph_ids"]).astype(np.int64)

    deg_out = np.maximum(np.bincount(src, minlength=N).astype(np.float64), 1.0)
    deg_in = np.maximum(np.bincount(dst, minlength=N).astype(np.float64), 1.0)
    iso = (1.0 / np.sqrt(deg_out)).astype(np.float32)   # inv_sqrt_out per node
    isi = (1.0 / np.sqrt(deg_in)).astype(np.float32)    # inv_sqrt_in per node
    cnt = np.maximum(np.bincount(gid, minlength=B).astype(np.float64), 1.0)
    cnt_inv = (1.0 / cnt).astype(np.float32).reshape(B, 1)

    # layer-0 gather table: m1 = nf * iso in the half-shard interleaved layout
    m1 = nf * iso[:, None]
    m1tab = np.zeros((cfg["NROWS"], C), np.float32)
    for k in range(NC):
        sh = m1[k * SHARD:(k + 1) * SHARD]          # [12500, C]
        m1tab[k * H:k * H + H] = sh[:H]
        m1tab[NC * H + k * H:NC * H + k * H + (SHARD - H)] = sh[H:]

    # softmax(lambdas) per layer, host-side (3x3 input params)
    lam64 = lam.astype(np.float64)
    ex = np.exp(lam64 - lam64.max(axis=1, keepdims=True))
    wsoft = (ex / ex.sum(axis=1, keepdims=True)).astype(np.float64)  # [L,3]

    # edge -> (core, block, slot, bucket, idx16); table row in interleaved layout
    core = dst // SHARD
    local = dst - core * SHARD
    blk = local // 128
    slot = (local % 128).astype(np.float32)
    sk = src // SHARD
    si = src - sk * SHARD
    row = np.where(si < H, sk * H + si, NC * H + sk * H + (si - H))
    buck = row // WIN
    idx16 = (row % WIN).astype(np.int16)

    # counts per (core, blk, buck)
    key = (core * NBLK + blk) * NBUCK + buck
    cnts = np.bincount(key, minlength=NC * NBLK * NBUCK).reshape(NC, NBLK, NBUCK)
    T = np.ceil(cnts.max(axis=0) / 128.0).astype(np.int64)  # [NBLK, NBUCK]
    # each phase of each block needs >=1 tile so PSUM gets a start matmul
    T[(T[:, 0] + T[:, 1]) == 0, 0] = 1
    T[(T[:, 2] + T[:, 3]) == 0, 2] = 1

    TQ = T.sum(axis=0)          # tiles per bucket stream
    EQ = TQ * 128               # padded edges per stream
    # slot offset of (blk) within stream q: running sum of T[:, q]
    off_blk = np.zeros((NBLK, NBUCK), np.int64)
    off_blk[1:] = np.cumsum(T[:-1] * 128, axis=0)

    order = np.lexsort((src, buck, blk, core))  # (core, blk, buck), src-sorted
    per_core = []
    for k in range(NC):
        sel = order[core[order] == k]
        bblk, bbuck = blk[sel], buck[sel]
        # position within (blk, buck) group
        grp = bblk * NBUCK + bbuck
        # stable order -> rank within group
        rank = np.zeros(len(sel), np.int64)
        if len(sel):
            gcnt = np.bincount(grp, minlength=NBLK * NBUCK)
            starts = np.concatenate([[0], np.cumsum(gcnt)[:-1]])
            # edges are sorted by grp already (lexsort by (blk,buck))
            rank = np.arange(len(sel)) - starts[grp]
        pos = off_blk[bblk, bbuck] + rank           # slot within stream bbuck
        d = {}
        for q in range(NBUCK):
            eq = int(EQ[q])
            idx_q = np.zeros(eq, np.int16)
            slot_q = -np.ones(eq, np.float32)
            m = bbuck == q
            idx_q[pos[m]] = idx16[sel[m]]
            slot_q[pos[m]] = slot[sel[m]]
            d[f"idxq{q}"] = np.tile(
                np.ascontiguousarray(idx_q.reshape(-1, 16).T), (8, 1))
            d[f"slotq{q}"] = np.ascontiguousarray(slot_q.reshape(-1, 128).T)
        # per-node columns for this shard (padded rows -> 1.0 / gid 0)
        lo, hi = k * SHARD, (k + 1) * SHARD
        pad = SHARD_PAD - SHARD
        isi_k = np.concatenate([isi[lo:hi], np.ones(pad, np.float32)])
        iso_k = np.concatenate([iso[lo:hi], np.ones(pad, np.float32)])
        d["inv_in_c"] = np.ascontiguousarray(isi_k.reshape(NBLK, 128).T)
        d["inv_out_c"] = np.ascontiguousarray(iso_k.reshape(NBLK, 128).T)
        gid_k = gid[lo:hi]
        G = np.zeros((SHARD_PAD, B), np.float32)
        G[np.arange(SHARD), gid_k] = 1.0
        G3 = G.reshape(NBLK, 128, B)
        d["g_oh"] = np.ascontiguousarray(G3.transpose(1, 0, 2)).reshape(128, NBLK * B)
        d["g_ohT"] = np.ascontiguousarray(G3.transpose(2, 0, 1)).reshape(B, NBLK * 128)
        per_core.append(d)

    consts = dict(
        iota=np.broadcast_to(np.arange(128, dtype=np.float32), (128, 128)).copy(),
        wmat=np.ascontiguousarray(W.transpose(1, 0, 2)).reshape(C, cfg["L"] * C),
        cnt_inv=cnt_inv,
        m1tab=m1tab,
        gamma=gamma, beta=beta,
    )
    gamma_trivial = bool(np.all(gamma == 1.0) and np.all(beta == 0.0))
    meta = dict(T=T, TQ=TQ, EQ=EQ, wsoft=wsoft, gamma_trivial=gamma_trivial,
                TMAX=int(T.max()))
    return meta, per_core, consts


def build_nc(cfg, meta):
    import concourse.bacc as bacc
    import concourse.bass as bass
    import concourse.mybir as mybir
    import concourse.tile as tile

    f32 = mybir.dt.float32
    GD = f32 if cfg["GD"] == "f32" else mybir.dt.bfloat16
    C, B, L = cfg["C"], cfg["B"], cfg["L"]
    NC, NBUCK, CH, IB = cfg["NCORES"], cfg["NBUCK"], cfg["CH"], cfg["IB"]
    NBLK, SHARD_PAD, NROWS, WIN = (cfg["NBLK"], cfg["SHARD_PAD"],
                                   cfg["NROWS"], cfg["WIN"])
    H, HBLK = cfg["H"], cfg["HBLK"]
    EPS, SLOPE, N = cfg["EPS"], cfg["SLOPE"], cfg["N"]
    T, TQ, EQ = meta["T"], meta["TQ"], meta["EQ"]
    wsoft, TMAX = meta["wsoft"], meta["TMAX"]
    gtriv = meta["gamma_trivial"]
    RG = [list(range(NC))]
    eq_ = mybir.AluOpType
    AF = mybir.ActivationFunctionType

    nc = bacc.Bacc("TRN2", target_bir_lowering=False, debug=False,
                   num_devices=NC, num_swdge_queues=4)

    # ---- DRAM tensors ----
    m1tab_t = nc.dram_tensor("m1tab", [NROWS, C], GD, kind="ExternalInput")
    out_t = nc.dram_tensor("out", [B, C], f32, kind="ExternalOutput")
    idx_t, slot_t = [], []
    for q in range(NBUCK):
        idx_t.append(nc.dram_tensor(f"idxq{q}", [128, int(EQ[q]) // 16],
                                    mybir.dt.int16, kind="ExternalInput"))
        slot_t.append(nc.dram_tensor(f"slotq{q}", [128, int(EQ[q]) // 128],
                                     GD, kind="ExternalInput"))
    invin_t = nc.dram_tensor("inv_in_c", [128, NBLK], f32, kind="ExternalInput")
    invout_t = nc.dram_tensor("inv_out_c", [128, NBLK], f32, kind="ExternalInput")
    goh_t = nc.dram_tensor("g_oh", [128, NBLK * B], f32, kind="ExternalInput")
    gohT_t = nc.dram_tensor("g_ohT", [B, NBLK * 128], f32, kind="ExternalInput")
    iota_t = nc.dram_tensor("iota", [128, 128], GD, kind="ExternalInput")
    wmat_t = nc.dram_tensor("wmat", [C, L * C], f32, kind="ExternalInput")
    cntinv_t = nc.dram_tensor("cnt_inv", [B, 1], f32, kind="ExternalInput")
    gamma_t = nc.dram_tensor("gamma", [L, C], f32, kind="ExternalInput")
    beta_t = nc.dram_tensor("beta", [L, C], f32, kind="ExternalInput")

    # per-layer tables: layer 0 reads m1tab; layers 1.. read the AG outputs
    mshA, mshB, mfA, mfB, stin, stout = [], [], [], [], [], []
    for l in range(L - 1):
        mshA.append(nc.dram_tensor(f"mshA{l}", [H, C], GD))
        mshB.append(nc.dram_tensor(f"mshB{l}", [H, C], GD))
        mfA.append(nc.dram_tensor(f"mfA{l}", [NC * H, C], GD,
                                  addr_space="Shared"))
        mfB.append(nc.dram_tensor(f"mfB{l}", [NC * H, C], GD,
                                  addr_space="Shared"))
    for l in range(L):
        stin.append(nc.dram_tensor(f"stin{l}", [B, 2 * C], f32))
        stout.append(nc.dram_tensor(f"stout{l}", [B, 2 * C], f32,
                                    addr_space="Shared"))
    embin = nc.dram_tensor("embin", [B, C], f32)
    embout = nc.dram_tensor("embout", [B, C], f32, addr_space="Shared")

    def tab_ap(l, q):
        if l == 0:
            return m1tab_t.ap()[q * WIN:(q + 1) * WIN, :]
        if q < 2:
            return mfA[l - 1].ap()[q * WIN:(q + 1) * WIN, :]
        return mfB[l - 1].ap()[(q - 2) * WIN:(q - 1) * WIN, :]

    with tile.TileContext(nc) as tc:
        with (
            tc.tile_pool(name="const", bufs=1) as cp,
            tc.tile_pool(name="big", bufs=1) as bigp,
            tc.tile_pool(name="gath", bufs=2) as gp,
            tc.tile_pool(name="gath3", bufs=3) as gp3,
            tc.tile_pool(name="work", bufs=3) as wp,
            tc.tile_pool(name="coef", bufs=1) as kp,
            tc.tile_pool(name="psum", bufs=2, space="PSUM") as pp,
            tc.tile_pool(name="psum1", bufs=1, space="PSUM") as pp1,
        ):
            # ---- resident constants ----
            iota = cp.tile([128, 128], GD)
            nc.sync.dma_start(iota[:], iota_t.ap())
            wm = cp.tile([C, L, C], f32)
            nc.sync.dma_start(wm[:], wmat_t.ap().rearrange("c (l k) -> c l k", l=L))
            goh = cp.tile([128, NBLK, B], f32)
            nc.sync.dma_start(goh[:], goh_t.ap().rearrange("p (b g) -> p b g", b=NBLK))
            goh16 = cp.tile([128, NBLK, B], GD)
            nc.vector.tensor_copy(goh16[:], goh[:])
            gTall = cp.tile([B, NBLK, 128], f32)
            nc.sync.dma_start(gTall[:],
                              gohT_t.ap().rearrange("g (b p) -> g b p", b=NBLK))
            invin = cp.tile([128, NBLK], f32)
            nc.sync.dma_start(invin[:], invin_t.ap())
            invout = cp.tile([128, NBLK], f32)
            nc.sync.dma_start(invout[:], invout_t.ap())
            invout02 = cp.tile([128, NBLK], f32)
            nc.vector.tensor_scalar_mul(invout02[:], invout[:], SLOPE)
            invout08 = cp.tile([128, NBLK], f32)
            nc.vector.tensor_scalar_mul(invout08[:], invout[:], 1.0 - SLOPE)
            cntinv = cp.tile([B, 1], f32)
            nc.sync.dma_start(cntinv[:], cntinv_t.ap())
            ones16 = cp.tile([B, 1], f32)
            nc.vector.memset(ones16[:], 1.0)
            ones1 = cp.tile([1, B], f32)
            nc.vector.memset(ones1[:], 1.0)
            ones1p = cp.tile([1, 128], f32)
            nc.vector.memset(ones1p[:], 1.0)
            eps128 = cp.tile([128, 1], f32)
            nc.vector.memset(eps128[:], EPS)
            gam = cp.tile([L, C], f32)
            nc.sync.dma_start(gam[:], gamma_t.ap())
            bet = cp.tile([L, C], f32)
            nc.sync.dma_start(bet[:], beta_t.ap())

            qctr = [0]  # unused placeholder (queues rewritten post-compile)
            # hbuf: h per block in pass A; overwritten in-place by pass B
            # with the next layer's m rows (h is dead once consumed).
            hbuf = bigp.tile([128, NBLK, C], GD)
            aggP = bigp.tile([C, NBLK, 128], GD)     # phase-1 partial aggT
            nm_arr = cp.tile([128, NBLK], f32)
            nv_arr = cp.tile([128, NBLK], f32)


            for l in range(L):
                w0, w1, w2 = [float(x) for x in wsoft[l]]
                gs_p = pp1.tile([B, C], f32, tag="gs")
                gss_p = pp1.tile([B, C], f32, tag="gss")
                cur = [0] * NBUCK          # consumed tiles per stream
                chunk_tiles = [None] * NBUCK
                chunk_id = [-1] * NBUCK
                ibatch_tiles = [None] * NBUCK
                ibatch_id = [-1] * NBUCK

                def load_ibatch(q, bi):
                    nt_b = min(IB * CH, int(TQ[q]) - bi * IB * CH)
                    it = gp.tile([128, IB * CH * 8], mybir.dt.int16,
                                 tag=f"i{q}", name=f"it{q}")
                    c0 = bi * IB * CH
                    nc.sync.dma_start(it[:, :nt_b * 8],
                                      idx_t[q].ap()[:, c0 * 8:(c0 + nt_b) * 8])
                    st = gp.tile([128, IB * CH], GD, tag=f"s{q}", name=f"st{q}")
                    nc.sync.dma_start(st[:, :nt_b],
                                      slot_t[q].ap()[:, c0:c0 + nt_b])
                    return (it, st)

                def issue_chunk(q, ci, l=l, ibatch_tiles=ibatch_tiles,
                                ibatch_id=ibatch_id):
                    bi = ci // IB
                    nbatch = (int(TQ[q]) + IB * CH - 1) // (IB * CH)
                    if ibatch_id[q] != bi:
                        prev = ibatch_tiles[q]
                        if ibatch_id[q] == bi - 1 and prev is not None and \
                                prev[1] is not None:
                            cur = prev[1]
                        else:
                            cur = load_ibatch(q, bi)
                        # prefetch the next idx batch so gathers don't stall
                        nxt = load_ibatch(q, bi + 1) if bi + 1 < nbatch else None
                        ibatch_tiles[q] = (cur, nxt)
                        ibatch_id[q] = bi
                    (it, st), _ = ibatch_tiles[q]
                    r = min(CH, int(TQ[q]) - ci * CH)
                    co = (ci % IB) * CH
                    gt = gp3.tile([128, CH, C], GD, tag=f"g{q}")
                    nc.gpsimd.dma_gather(
                        gt[:, :r, :], tab_ap(l, q),
                        it[:, co * 8:(co + r) * 8],
                        r * 128, r * 128, C, queue_num=0)
                    S = gp.tile([128, CH, 128], GD, tag=f"S{q}", name=f"S{q}")
                    nc.vector.tensor_tensor(
                        out=S[:, :r, :],
                        in0=iota[:].unsqueeze(1).broadcast_to([128, r, 128]),
                        in1=st[:, co:co + r].unsqueeze(2)
                            .broadcast_to([128, r, 128]),
                        op=eq_.is_equal)
                    return (gt, S)

                def consume(b, qs, aggT_p):
                    ntot = int(sum(T[b, q] for q in qs))
                    done = 0
                    for q in qs:
                        nt = int(T[b, q])
                        t0 = cur[q]
                        cur[q] += nt
                        while nt > 0:
                            ci = t0 // CH
                            if chunk_id[q] != ci:
                                chunk_tiles[q] = issue_chunk(q, ci)
                                chunk_id[q] = ci
                            col = t0 % CH
                            r = min(nt, CH - col)
                            gt, S = chunk_tiles[q]
                            for j in range(r):
                                nc.tensor.matmul(
                                    aggT_p[:], gt[:, col + j, :], S[:, col + j, :],
                                    start=(done == 0), stop=(done == ntot - 1))
                                done += 1
                            t0 += r
                            nt -= r

                # ---------------- PASS A phase 1: buckets 0,1 ----------------
                scope1 = nc.named_scope(f"passA1_{l}"); scope1.__enter__()
                for b in range(NBLK):
                    aggT_p = pp.tile([C, 128], f32, tag="aggT")
                    consume(b, (0, 1), aggT_p)
                    nc.scalar.activation(aggP[:, b, :], aggT_p[:], AF.Copy)
                scope1.__exit__(None, None, None)

                # ---------------- PASS A phase 2: buckets 2,3 ----------------
                scope2 = nc.named_scope(f"passA2_{l}"); scope2.__enter__()
                for b in range(NBLK):
                    aggT_p = pp.tile([C, 128], f32, tag="aggT")
                    consume(b, (2, 3), aggT_p)
                    aggT_s = wp.tile([C, 128], f32, tag="aggTs")
                    nc.vector.tensor_tensor(out=aggT_s[:], in0=aggT_p[:],
                                            in1=aggP[:, b, :], op=eq_.add)
                    h_p = pp.tile([128, C], f32, tag="hp")
                    nc.tensor.matmul(h_p[:], aggT_s[:], wm[:, l, :],
                                     start=True, stop=True)
                    nc.scalar.activation(hbuf[:, b, :], h_p[:], AF.Copy,
                                         scale=invin[:, b:b + 1],
                                         accum_out=nm_arr[:, b:b + 1])
                    h2 = wp.tile([128, C], GD, tag="h2")
                    nc.scalar.activation(h2[:], hbuf[:, b, :], AF.Square,
                                         accum_out=nv_arr[:, b:b + 1])
                    nc.tensor.matmul(gs_p[:], goh16[:, b, :], hbuf[:, b, :],
                                     start=(b == 0), stop=(b == NBLK - 1))
                    nc.tensor.matmul(gss_p[:], goh16[:, b, :], h2[:],
                                     start=(b == 0), stop=(b == NBLK - 1))
                scope2.__exit__(None, None, None)

                scopeS = nc.named_scope(f"stats_{l}"); scopeS.__enter__()
                # ---- stats AllReduce ----
                sts = kp.tile([B, 2 * C], f32, tag="sts")
                nc.vector.tensor_copy(sts[:, 0:C], gs_p[:])
                nc.vector.tensor_copy(sts[:, C:2 * C], gss_p[:])
                nc.sync.dma_start(stin[l].ap(), sts[:])
                nc.gpsimd.collective_compute(
                    "AllReduce", eq_.add, ins=[stin[l].ap()],
                    outs=[stout[l].ap()], replica_groups=RG)
                gst = kp.tile([B, 2 * C], f32, tag="gst")
                nc.sync.dma_start(gst[:], stout[l].ap())
                gs, gss = gst[:, 0:C], gst[:, C:2 * C]

                # ---- coefficients AB = [A16 | B16] (bf16) ----
                gm = kp.tile([B, C], f32, tag="gm")
                nc.vector.tensor_scalar_mul(gm[:], gs, cntinv[:])
                gv = kp.tile([B, C], f32, tag="gv")
                nc.vector.tensor_scalar_mul(gv[:], gss, cntinv[:])
                tmp16 = kp.tile([B, C], f32, tag="tmp16")
                nc.vector.tensor_tensor(out=tmp16[:], in0=gm[:], in1=gm[:], op=eq_.mult)
                nc.vector.tensor_tensor(out=gv[:], in0=gv[:], in1=tmp16[:], op=eq_.subtract)
                nc.scalar.activation(gv[:], gv[:], AF.Sqrt, bias=eps128[0:B, :])
                igv = kp.tile([B, C], f32, tag="igv")
                nc.vector.reciprocal(igv[:], gv[:])
                bs_p = pp.tile([1, 2 * C], f32, tag="aggT")
                nc.tensor.matmul(bs_p[:], ones16[:], gst[:], start=True, stop=True)
                bm = kp.tile([1, C], f32, tag="bm")
                nc.vector.tensor_scalar_mul(bm[:], bs_p[:, 0:C], 1.0 / N)
                bv = kp.tile([1, C], f32, tag="bv")
                nc.vector.tensor_scalar_mul(bv[:], bs_p[:, C:2 * C], 1.0 / N)
                tmp1 = kp.tile([1, C], f32, tag="tmp1")
                nc.vector.tensor_tensor(out=tmp1[:], in0=bm[:], in1=bm[:], op=eq_.mult)
                nc.vector.tensor_tensor(out=bv[:], in0=bv[:], in1=tmp1[:], op=eq_.subtract)
                nc.scalar.activation(bv[:], bv[:], AF.Sqrt, bias=eps128[0:1, :])
                ibv = kp.tile([1, C], f32, tag="ibv")
                nc.vector.reciprocal(ibv[:], bv[:])
                # broadcast [1,C] rows to B partitions via K=1 matmul
                ibv_p = pp.tile([B, C], f32, tag="aggT")
                nc.tensor.matmul(ibv_p[:], ones1[:], ibv[:], start=True, stop=True)
                bmibv = kp.tile([1, C], f32, tag="bmibv")
                nc.vector.tensor_tensor(out=bmibv[:], in0=bm[:], in1=ibv[:], op=eq_.mult)
                bmibv_p = pp.tile([B, C], f32, tag="hp")
                nc.tensor.matmul(bmibv_p[:], ones1[:], bmibv[:], start=True, stop=True)
                AB = kp.tile([B, 2 * C], f32, tag="AB")
                A16, B16 = AB[:, 0:C], AB[:, C:2 * C]
                nc.vector.tensor_scalar_mul(A16, igv[:], w1)
                t16b = kp.tile([B, C], f32, tag="t16b")
                nc.vector.tensor_scalar_mul(t16b[:], ibv_p[:], w0)
                nc.vector.tensor_tensor(out=A16, in0=A16, in1=t16b[:], op=eq_.add)
                nc.vector.tensor_tensor(out=B16, in0=gm[:], in1=igv[:], op=eq_.mult)
                nc.vector.tensor_scalar_mul(B16, B16, w1)
                nc.vector.tensor_scalar_mul(t16b[:], bmibv_p[:], w0)
                nc.vector.tensor_tensor(out=B16, in0=B16, in1=t16b[:], op=eq_.add)

                # ---- per-node coefficients: a_n = w2*invn, bn_neg = -nm*a_n ----
                nmm = kp.tile([128, NBLK], f32, tag="nmm")
                nc.vector.tensor_scalar_mul(nmm[:], nm_arr[:], 1.0 / C)
                nvm = kp.tile([128, NBLK], f32, tag="nvm")
                nc.vector.tensor_scalar_mul(nvm[:], nv_arr[:], 1.0 / C)
                nm2 = kp.tile([128, NBLK], f32, tag="nm2")
                nc.vector.tensor_tensor(out=nm2[:], in0=nmm[:], in1=nmm[:], op=eq_.mult)
                nc.vector.tensor_tensor(out=nvm[:], in0=nvm[:], in1=nm2[:], op=eq_.subtract)
                nc.scalar.activation(nvm[:], nvm[:], AF.Sqrt, bias=eps128[:])
                invn = kp.tile([128, NBLK], f32, tag="invn")
                nc.vector.reciprocal(invn[:], nvm[:])
                a_n = kp.tile([128, NBLK], f32, tag="a_n")
                nc.vector.tensor_scalar_mul(a_n[:], invn[:], w2)
                bn_neg = kp.tile([128, NBLK], f32, tag="bn_neg")
                nc.vector.tensor_tensor(out=bn_neg[:], in0=nmm[:], in1=a_n[:], op=eq_.mult)
                nc.vector.tensor_scalar_mul(bn_neg[:], bn_neg[:], -1.0)

                scopeS.__exit__(None, None, None)
                if l == L - 1:
                    emb_p = pp1.tile([B, C], f32, tag="emb")

                # ---------------- PASS B ----------------
                scopeB = nc.named_scope(f"passB_{l}"); scopeB.__enter__()
                for b in range(NBLK):
                    AB_p = pp.tile([128, 2 * C], f32, tag="aggT")
                    nc.tensor.matmul(AB_p[:], gTall[:, b, :], AB[:],
                                     start=True, stop=True)
                    h = hbuf[:, b, :]
                    # w = a_n*h - b_n   (ACT, per-node scale/bias)
                    w_t = wp.tile([128, C], f32, tag="w_t")
                    nc.scalar.activation(w_t[:], h, AF.Identity,
                                         scale=a_n[:, b:b + 1],
                                         bias=bn_neg[:, b:b + 1])
                    # v = h*A_p - B_p  (DVE, PSUM operands)
                    v = wp.tile([128, C], f32, tag="v")
                    nc.vector.tensor_tensor(out=v[:], in0=h, in1=AB_p[:, 0:C], op=eq_.mult)
                    nc.vector.tensor_tensor(out=v[:], in0=v[:], in1=AB_p[:, C:2 * C],
                                            op=eq_.subtract)
                    u = wp.tile([128, C], f32, tag="u")
                    nc.vector.tensor_tensor(out=u[:], in0=v[:], in1=w_t[:], op=eq_.add)
                    if not gtriv:
                        gam_p = pp.tile([128, C], f32, tag="hp")
                        nc.tensor.matmul(gam_p[:], ones1p[:], gam[l:l + 1, :],
                                         start=True, stop=True)
                        bet_p = pp.tile([128, C], f32, tag="hp2")
                        nc.tensor.matmul(bet_p[:], ones1p[:], bet[l:l + 1, :],
                                         start=True, stop=True)
                        nc.vector.tensor_tensor(out=u[:], in0=u[:], in1=gam_p[:], op=eq_.mult)
                        nc.vector.tensor_tensor(out=u[:], in0=u[:], in1=bet_p[:], op=eq_.add)
                    if l < L - 1:
                        # m = leaky(u)*invout = SLOPE*invout*u + (1-SLOPE)*relu(u*invout)
                        r_t = wp.tile([128, C], f32, tag="r_t")
                        nc.scalar.activation(r_t[:], u[:], AF.Relu,
                                             scale=invout08[:, b:b + 1])
                        nc.vector.tensor_scalar(u[:], u[:],
                                                invout02[:, b:b + 1], None,
                                                eq_.mult)
                        nc.vector.tensor_tensor(out=hbuf[:, b, :], in0=u[:],
                                                in1=r_t[:], op=eq_.add)
                        if b == HBLK - 1:
                            nc.sync.dma_start(
                                mshA[l].ap().rearrange("(b p) c -> p b c", p=128),
                                hbuf[:, 0:HBLK, :])
                            nc.gpsimd.collective_compute(
                                "AllGather", eq_.bypass, ins=[mshA[l].ap()],
                                outs=[mfA[l].ap()], replica_groups=RG)
                        elif b == NBLK - 1:
                            nc.sync.dma_start(
                                mshB[l].ap().rearrange("(b p) c -> p b c", p=128),
                                hbuf[:, HBLK:NBLK, :])
                            nc.gpsimd.collective_compute(
                                "AllGather", eq_.bypass, ins=[mshB[l].ap()],
                                outs=[mfB[l].ap()], replica_groups=RG)
                    else:
                        r_t = wp.tile([128, C], f32, tag="r_t")
                        nc.scalar.activation(r_t[:], u[:], AF.Relu,
                                             scale=1.0 - SLOPE)
                        nc.vector.tensor_scalar_mul(u[:], u[:], SLOPE)
                        lu = wp.tile([128, C], f32, tag="lu")
                        nc.vector.tensor_tensor(out=lu[:], in0=u[:],
                                                in1=r_t[:], op=eq_.add)
                        nc.tensor.matmul(emb_p[:], goh[:, b, :], lu[:],
                                         start=(b == 0), stop=(b == NBLK - 1))
                scopeB.__exit__(None, None, None)

            # ---- readout ----
            embs = kp.tile([B, C], f32, tag="embs")
            nc.vector.tensor_copy(embs[:], emb_p[:])
            nc.sync.dma_start(embin.ap(), embs[:])
            nc.gpsimd.collective_compute(
                "AllReduce", eq_.add, ins=[embin.ap()], outs=[embout.ap()],
                replica_groups=RG)
            embg = kp.tile([B, C], f32, tag="embg")
            nc.sync.dma_start(embg[:], embout.ap())
            nc.vector.tensor_scalar_mul(embg[:], embg[:], cntinv[:])
            embg2 = kp.tile([B, C], f32, tag="embg2")
            nc.vector.tensor_scalar_mul(embg2[:], embg[:], SLOPE)
            nc.vector.tensor_tensor(out=embg[:], in0=embg[:], in1=embg2[:], op=eq_.max)
            nc.sync.dma_start(out_t.ap(), embg[:])

    nc.compile()
    _fixup_swdge_queues(nc)
    return nc


def _fixup_swdge_queues(nc):
    """Rewrite gather queue_num in scheduled (module) order.

    Tile assigns DMASW completion-sem lanes round-robin (mod 8) over
    Pool-engine DMA instructions in module order, and the ucode requires a
    sem lane to be incremented from a single SWDGE queue. Setting
    queue = lane % 4 in the same walk order satisfies that for any schedule
    while keeping all 4 queues busy.
    """
    import concourse.mybir as mybir
    pool_dma = ("InstDMAGatherAnt", "InstDMAScatterAddAnt",
                "InstKVWritebackAnt", "InstPagedWritebackAnt")
    lane = 0
    for f in nc.m.functions:
        for bb in f.blocks:
            for inst in bb.instructions:
                cls = type(inst).__name__
                is_pool_dma = cls in pool_dma or (
                    cls in ("InstDMACopy", "InstTensorLoad", "InstTensorSave")
                    and getattr(inst, "engine", None) == mybir.EngineType.Pool)
                if is_pool_dma:
                    inst.queue_num = lane % 4
                    lane += 1


def make_in_maps(cfg, per_core, consts):
    import ml_dtypes
    GD_np = np.float32 if cfg["GD"] == "f32" else ml_dtypes.bfloat16
    base = dict(
        m1tab=consts["m1tab"].astype(GD_np),
        iota=consts["iota"].astype(GD_np), wmat=consts["wmat"],
        cnt_inv=consts["cnt_inv"], gamma=consts["gamma"], beta=consts["beta"],
    )
    in_maps = []
    for k in range(cfg["NCORES"]):
        d = dict(base)
        for kk, vv in per_core[k].items():
            if kk.startswith("slotq"):
                vv = vv.astype(GD_np)
            d[kk] = vv
        in_maps.append(d)
    return in_maps


_BUILD_CACHE = {}


def _build_cached(cfg, meta):
    key = (tuple(sorted((k, str(v)) for k, v in cfg.items())),
           meta["T"].tobytes(), meta["wsoft"].tobytes(), meta["gamma_trivial"])
    if key not in _BUILD_CACHE:
        _BUILD_CACHE[key] = build_nc(cfg, meta)
    return _BUILD_CACHE[key]


def run_cfg(cfg, inputs, trace=False, verbose=False):
    import time
    from concourse.bass_utils import run_bass_kernel_spmd
    t0 = time.time()
    cfg = _derive(cfg)
    meta, per_core, consts = prep_host(inputs, cfg)
    t1 = time.time()
    nc = _build_cached(cfg, meta)
    t2 = time.time()
    in_maps = make_in_maps(cfg, per_core, consts)
    res = run_bass_kernel_spmd(nc, in_maps, list(range(cfg["NCORES"])),
                               trace=trace)
    t3 = time.time()
    if verbose:
        print(f"[timing] prep={t1-t0:.1f}s build+compile={t2-t1:.1f}s "
              f"run={t3-t2:.1f}s", flush=True)
    return res.results[0]["out"].astype(np.float32), res


def kernel(**inputs):
    out, _ = run_cfg(_cfg_real(), inputs)
    return out


# revision 39
# speedup vs baseline: 1.5806x; 1.1449x over previous
"""Trainium2 Bass kernel for nn_CONVMGEmbedder (3-layer GraphConv + UnitedNorm + readout).

Strategy: dst-sharded graph partition over 8 NeuronCores.
- Node shard k = rows [k*12500, (k+1)*12500), padded to 12544 (98 blocks of 128).
- Edges live on their dst-owner core, grouped by (dst block, src bucket),
  padded to a global (SPMD-uniform) tile table.
- Table layout is half-shard interleaved: table row of node (k, i) =
  k*H + i for i < H (= 6272), and NC*H + k*H + (i-H) otherwise. The
  inter-layer AllGather is split in two (mfullA = all cores' first half-
  shards -> src buckets 0-1, mfullB -> buckets 2-3), so AG(A) overlaps the
  tail of pass B and AG(B) overlaps phase-1 gathers of the next layer.
- Pass A runs two phases per layer: phase 1 consumes buckets {0,1} into
  per-block PSUM, flushed to an SBUF partial (bf16); phase 2 consumes
  buckets {2,3} and combines.
- Aggregation: dma_gather of m[src] rows (1024-row calls - the HW limit -
  rotated across the 4 SWDGE queues), one-hot S tiles (DVE is_equal),
  PE matmuls aggT += m_e^T @ S.
- h = (aggT.T @ W) * inv_sqrt_in (ACT copy w/ per-node scale, fused row-sum).
- UnitedNorm via per-graph coefficient tile AB (one fused matmul per block)
  + per-node scalars on ACT; leaky = SLOPE*x + (1-SLOPE)*relu(x) on ACT/DVE
  with the inv_sqrt_out scale folded in (positive homogeneity).
- Readout: G^T @ h3 accumulated in PSUM, AllReduce, /cnt, leaky.
"""
import math
import os
import sys

sys.path.insert(0, "/opt/trn_rl_repo")

import numpy as np


def _cfg_real():
    return dict(
        N=100000, E=1600000, C=128, B=16, L=3, NCORES=8,
        NBUCK=4, CH=8, IB=8, GD="bf16",
    )


def _derive(cfg):
    c = dict(cfg)
    c["SHARD"] = c["N"] // c["NCORES"]
    c["NBLK"] = (c["SHARD"] + 127) // 128
    c["SHARD_PAD"] = c["NBLK"] * 128
    c["NROWS"] = c["NCORES"] * c["SHARD_PAD"]
    assert c["NBLK"] % 2 == 0
    c["H"] = c["SHARD_PAD"] // 2          # half-shard rows
    c["HBLK"] = c["NBLK"] // 2
    assert c["NROWS"] % c["NBUCK"] == 0
    c["WIN"] = c["NROWS"] // c["NBUCK"]
    assert c["WIN"] <= 32768, c["WIN"]
    assert c["NCORES"] * c["H"] == 2 * c["WIN"]
    c["EPS"] = 1e-5
    c["SLOPE"] = 0.2
    return c


def prep_host(inputs, cfg):
    """Pure-numpy sharding prep: degrees, edge reorder, tile tables, constants."""
    N, E, C, B = cfg["N"], cfg["E"], cfg["C"], cfg["B"]
    NC, NBUCK = cfg["NCORES"], cfg["NBUCK"]
    SHARD, NBLK, H = cfg["SHARD"], cfg["NBLK"], cfg["H"]
    SHARD_PAD, WIN = cfg["SHARD_PAD"], cfg["WIN"]

    nf = np.asarray(inputs["node_feats"], np.float32)
    W = np.asarray(inputs["W"], np.float32)
    gamma = np.asarray(inputs["gamma"], np.float32)
    beta = np.asarray(inputs["beta"], np.float32)
    lam = np.asarray(inputs["lambdas"], np.float32)
    src = np.asarray(inputs["src"]).astype(np.int64)
    dst = np.asarray(inputs["dst"]).astype(np.int64)
    gid = np.asarray(inputs["graph_ids"]).astype(np.int64)

    deg_out = np.maximum(np.bincount(src, minlength=N).astype(np.float64), 1.0)
    deg_in = np.maximum(np.bincount(dst, minlength=N).astype(np.float64), 1.0)
    iso = (1.0 / np.sqrt(deg_out)).astype(np.float32)   # inv_sqrt_out per node
    isi = (1.0 / np.sqrt(deg_in)).astype(np.float32)    # inv_sqrt_in per node
    cnt = np.maximum(np.bincount(gid, minlength=B).astype(np.float64), 1.0)
    cnt_inv = (1.0 / cnt).astype(np.float32).reshape(B, 1)

    # layer-0 gather table: m1 = nf * iso in the half-shard interleaved layout
    m1 = nf * iso[:, None]
    m1tab = np.zeros((cfg["NROWS"], C), np.float32)
    for k in range(NC):
        sh = m1[k * SHARD:(k + 1) * SHARD]          # [12500, C]
        m1tab[k * H:k * H + H] = sh[:H]
        m1tab[NC * H + k * H:NC * H + k * H + (SHARD - H)] = sh[H:]

    # softmax(lambdas) per layer, host-side (3x3 input params)
    lam64 = lam.astype(np.float64)
    ex = np.exp(lam64 - lam64.max(axis=1, keepdims=True))
    wsoft = (ex / ex.sum(axis=1, keepdims=True)).astype(np.float64)  # [L,3]

    # edge -> (core, block, slot, bucket, idx16); table row in interleaved layout
    core = dst // SHARD
    local = dst - core * SHARD
    blk = local // 128
    slot = (local % 128).astype(np.float32)
    sk = src // SHARD
    si = src - sk * SHARD
    row = np.where(si < H, sk * H + si, NC * H + sk * H + (si - H))
    buck = row // WIN
    idx16 = (row % WIN).astype(np.int16)

    # counts per (core, blk, buck)
    key = (core * NBLK + blk) * NBUCK + buck
    cnts = np.bincount(key, minlength=NC * NBLK * NBUCK).reshape(NC, NBLK, NBUCK)
    T = np.ceil(cnts.max(axis=0) / 128.0).astype(np.int64)  # [NBLK, NBUCK]
    # each phase of each block needs >=1 tile so PSUM gets a start matmul
    T[(T[:, 0] + T[:, 1]) == 0, 0] = 1
    T[(T[:, 2] + T[:, 3]) == 0, 2] = 1

    TQ = T.sum(axis=0)          # tiles per bucket stream
    EQ = TQ * 128               # padded edges per stream
    # slot offset of (blk) within stream q: running sum of T[:, q]
    off_blk = np.zeros((NBLK, NBUCK), np.int64)
    off_blk[1:] = np.cumsum(T[:-1] * 128, axis=0)

    order = np.lexsort((src, buck, blk, core))  # (core, blk, buck), src-sorted
    per_core = []
    for k in range(NC):
        sel = order[core[order] == k]
        bblk, bbuck = blk[sel], buck[sel]
        # position within (blk, buck) group
        grp = bblk * NBUCK + bbuck
        # stable order -> rank within group
        rank = np.zeros(len(sel), np.int64)
        if len(sel):
            gcnt = np.bincount(grp, minlength=NBLK * NBUCK)
            starts = np.concatenate([[0], np.cumsum(gcnt)[:-1]])
            # edges are sorted by grp already (lexsort by (blk,buck))
            rank = np.arange(len(sel)) - starts[grp]
        pos = off_blk[bblk, bbuck] + rank           # slot within stream bbuck
        d = {}
        for q in range(NBUCK):
            eq = int(EQ[q])
            idx_q = np.zeros(eq, np.int16)
            slot_q = -np.ones(eq, np.float32)
            m = bbuck == q
            idx_q[pos[m]] = idx16[sel[m]]
            slot_q[pos[m]] = slot[sel[m]]
            d[f"idxq{q}"] = np.tile(
                np.ascontiguousarray(idx_q.reshape(-1, 16).T), (8, 1))
            d[f"slotq{q}"] = np.ascontiguousarray(slot_q.reshape(-1, 128).T)
        # per-node columns for this shard (padded rows -> 1.0 / gid 0)
        lo, hi = k * SHARD, (k + 1) * SHARD
        pad = SHARD_PAD - SHARD
        isi_k = np.concatenate([isi[lo:hi], np.ones(pad, np.float32)])
        iso_k = np.concatenate([iso[lo:hi], np.ones(pad, np.float32)])
        d["inv_in_c"] = np.ascontiguousarray(isi_k.reshape(NBLK, 128).T)
        d["inv_out_c"] = np.ascontiguousarray(iso_k.reshape(NBLK, 128).T)
        gid_k = gid[lo:hi]
        G = np.zeros((SHARD_PAD, B), np.float32)
        G[np.arange(SHARD), gid_k] = 1.0
        G3 = G.reshape(NBLK, 128, B)
        d["g_oh"] = np.ascontiguousarray(G3.transpose(1, 0, 2)).reshape(128, NBLK * B)
        d["g_ohT"] = np.ascontiguousarray(G3.transpose(2, 0, 1)).reshape(B, NBLK * 128)
        per_core.append(d)

    consts = dict(
        iota=np.broadcast_to(np.arange(128, dtype=np.float32), (128, 128)).copy(),
        wmat=np.ascontiguousarray(W.transpose(1, 0, 2)).reshape(C, cfg["L"] * C),
        cnt_inv=cnt_inv,
        m1tab=m1tab,
        gamma=gamma, beta=beta,
    )
    gamma_trivial = bool(np.all(gamma == 1.0) and np.all(beta == 0.0))
    meta = dict(T=T, TQ=TQ, EQ=EQ, wsoft=wsoft, gamma_trivial=gamma_trivial,
                TMAX=int(T.max()))
    return meta, per_core, consts


def build_nc(cfg, meta):
    import concourse.bacc as bacc
    import concourse.bass as bass
    import concourse.mybir as mybir
    import concourse.tile as tile

    f32 = mybir.dt.float32
    GD = f32 if cfg["GD"] == "f32" else mybir.dt.bfloat16
    C, B, L = cfg["C"], cfg["B"], cfg["L"]
    NC, NBUCK, CH, IB = cfg["NCORES"], cfg["NBUCK"], cfg["CH"], cfg["IB"]
    NBLK, SHARD_PAD, NROWS, WIN = (cfg["NBLK"], cfg["SHARD_PAD"],
                                   cfg["NROWS"], cfg["WIN"])
    H, HBLK = cfg["H"], cfg["HBLK"]
    EPS, SLOPE, N = cfg["EPS"], cfg["SLOPE"], cfg["N"]
    T, TQ, EQ = meta["T"], meta["TQ"], meta["EQ"]
    wsoft, TMAX = meta["wsoft"], meta["TMAX"]
    gtriv = meta["gamma_trivial"]
    RG = [list(range(NC))]
    eq_ = mybir.AluOpType
    AF = mybir.ActivationFunctionType

    nc = bacc.Bacc("TRN2", target_bir_lowering=False, debug=False,
                   num_devices=NC, num_swdge_queues=4)

    # ---- DRAM tensors ----
    m1tab_t = nc.dram_tensor("m1tab", [NROWS, C], GD, kind="ExternalInput")
    out_t = nc.dram_tensor("out", [B, C], f32, kind="ExternalOutput")
    idx_t, slot_t = [], []
    for q in range(NBUCK):
        idx_t.append(nc.dram_tensor(f"idxq{q}", [128, int(EQ[q]) // 16],
                                    mybir.dt.int16, kind="ExternalInput"))
        slot_t.append(nc.dram_tensor(f"slotq{q}", [128, int(EQ[q]) // 128],
                                     GD, kind="ExternalInput"))
    invin_t = nc.dram_tensor("inv_in_c", [128, NBLK], f32, kind="ExternalInput")
    invout_t = nc.dram_tensor("inv_out_c", [128, NBLK], f32, kind="ExternalInput")
    goh_t = nc.dram_tensor("g_oh", [128, NBLK * B], f32, kind="ExternalInput")
    gohT_t = nc.dram_tensor("g_ohT", [B, NBLK * 128], f32, kind="ExternalInput")
    iota_t = nc.dram_tensor("iota", [128, 128], GD, kind="ExternalInput")
    wmat_t = nc.dram_tensor("wmat", [C, L * C], f32, kind="ExternalInput")
    cntinv_t = nc.dram_tensor("cnt_inv", [B, 1], f32, kind="ExternalInput")
    gamma_t = nc.dram_tensor("gamma", [L, C], f32, kind="ExternalInput")
    beta_t = nc.dram_tensor("beta", [L, C], f32, kind="ExternalInput")

    # per-layer tables: layer 0 reads m1tab; layers 1.. read the AG outputs
    mshA, mshB, mfA, mfB, stin, stout = [], [], [], [], [], []
    for l in range(L - 1):
        mshA.append(nc.dram_tensor(f"mshA{l}", [H, C], GD))
        mshB.append(nc.dram_tensor(f"mshB{l}", [H, C], GD))
        mfA.append(nc.dram_tensor(f"mfA{l}", [NC * H, C], GD,
                                  addr_space="Shared"))
        mfB.append(nc.dram_tensor(f"mfB{l}", [NC * H, C], GD,
                                  addr_space="Shared"))
    stinA, stoutA = [], []
    for l in range(L):
        stin.append(nc.dram_tensor(f"stin{l}", [B, 2 * C], f32))
        stout.append(nc.dram_tensor(f"stout{l}", [B, 2 * C], f32,
                                    addr_space="Shared"))
        stinA.append(nc.dram_tensor(f"stinA{l}", [B, 2 * C], f32))
        stoutA.append(nc.dram_tensor(f"stoutA{l}", [B, 2 * C], f32,
                                     addr_space="Shared"))
    embin = nc.dram_tensor("embin", [B, C], f32)
    embout = nc.dram_tensor("embout", [B, C], f32, addr_space="Shared")

    def tab_ap(l, q):
        if l == 0:
            return m1tab_t.ap()[q * WIN:(q + 1) * WIN, :]
        if q < 2:
            return mfA[l - 1].ap()[q * WIN:(q + 1) * WIN, :]
        return mfB[l - 1].ap()[(q - 2) * WIN:(q - 1) * WIN, :]

    with tile.TileContext(nc) as tc:
        with (
            tc.tile_pool(name="const", bufs=1) as cp,
            tc.tile_pool(name="big", bufs=1) as bigp,
            tc.tile_pool(name="gath", bufs=2) as gp,
            tc.tile_pool(name="gathS", bufs=3) as gpS,
            tc.tile_pool(name="gath3", bufs=4) as gp3,
            tc.tile_pool(name="work", bufs=3) as wp,
            tc.tile_pool(name="coef", bufs=1) as kp,
            tc.tile_pool(name="psum", bufs=2, space="PSUM") as pp,
            tc.tile_pool(name="psumA", bufs=3, space="PSUM") as ppA,
            tc.tile_pool(name="psum1", bufs=1, space="PSUM") as pp1,
        ):
            # ---- resident constants ----
            iota = cp.tile([128, 128], GD)
            nc.sync.dma_start(iota[:], iota_t.ap())
            wm = cp.tile([C, L, C], f32)
            nc.sync.dma_start(wm[:], wmat_t.ap().rearrange("c (l k) -> c l k", l=L))
            goh = cp.tile([128, NBLK, B], f32)
            nc.sync.dma_start(goh[:], goh_t.ap().rearrange("p (b g) -> p b g", b=NBLK))
            goh16 = cp.tile([128, NBLK, B], GD)
            nc.vector.tensor_copy(goh16[:], goh[:])
            gTall = cp.tile([B, NBLK, 128], f32)
            nc.sync.dma_start(gTall[:],
                              gohT_t.ap().rearrange("g (b p) -> g b p", b=NBLK))
            invin = cp.tile([128, NBLK], f32)
            nc.sync.dma_start(invin[:], invin_t.ap())
            invout = cp.tile([128, NBLK], f32)
            nc.sync.dma_start(invout[:], invout_t.ap())
            invout02 = cp.tile([128, NBLK], f32)
            nc.vector.tensor_scalar_mul(invout02[:], invout[:], SLOPE)
            invout08 = cp.tile([128, NBLK], f32)
            nc.vector.tensor_scalar_mul(invout08[:], invout[:], 1.0 - SLOPE)
            cntinv = cp.tile([B, 1], f32)
            nc.sync.dma_start(cntinv[:], cntinv_t.ap())
            ones16 = cp.tile([B, 1], f32)
            nc.vector.memset(ones16[:], 1.0)
            ones1 = cp.tile([1, B], f32)
            nc.vector.memset(ones1[:], 1.0)
            ones1p = cp.tile([1, 128], f32)
            nc.vector.memset(ones1p[:], 1.0)
            eps128 = cp.tile([128, 1], f32)
            nc.vector.memset(eps128[:], EPS)
            gam = cp.tile([L, C], f32)
            nc.sync.dma_start(gam[:], gamma_t.ap())
            bet = cp.tile([L, C], f32)
            nc.sync.dma_start(bet[:], beta_t.ap())

            qctr = [0]  # unused placeholder (queues rewritten post-compile)
            # hbuf: h per block in pass A; overwritten in-place by pass B
            # with the next layer's m rows (h is dead once consumed).
            hbuf = bigp.tile([128, NBLK, C], GD)
            aggP = bigp.tile([C, NBLK, 128], GD)     # phase-1 partial aggT
            nm_arr = cp.tile([128, NBLK], f32)
            nv_arr = cp.tile([128, NBLK], f32)


            for l in range(L):
                w0, w1, w2 = [float(x) for x in wsoft[l]]
                gs_p = pp1.tile([B, C], f32, tag="gs")
                gss_p = pp1.tile([B, C], f32, tag="gss")
                cur = [0] * NBUCK          # consumed tiles per stream
                chunk_tiles = [None] * NBUCK
                chunk_id = [-1] * NBUCK
                ibatch_tiles = [None] * NBUCK
                ibatch_id = [-1] * NBUCK

                def load_ibatch(q, bi):
                    nt_b = min(IB * CH, int(TQ[q]) - bi * IB * CH)
                    it = gp.tile([128, IB * CH * 8], mybir.dt.int16,
                                 tag=f"i{q}", name=f"it{q}")
                    c0 = bi * IB * CH
                    nc.sync.dma_start(it[:, :nt_b * 8],
                                      idx_t[q].ap()[:, c0 * 8:(c0 + nt_b) * 8])
                    st = gp.tile([128, IB * CH], GD, tag=f"s{q}", name=f"st{q}")
                    nc.sync.dma_start(st[:, :nt_b],
                                      slot_t[q].ap()[:, c0:c0 + nt_b])
                    return (it, st)

                def issue_chunk(q, ci, l=l, ibatch_tiles=ibatch_tiles,
                                ibatch_id=ibatch_id):
                    bi = ci // IB
                    nbatch = (int(TQ[q]) + IB * CH - 1) // (IB * CH)
                    if ibatch_id[q] != bi:
                        prev = ibatch_tiles[q]
                        if ibatch_id[q] == bi - 1 and prev is not None and \
                                prev[1] is not None:
                            cur = prev[1]
                        else:
                            cur = load_ibatch(q, bi)
                        # prefetch the next idx batch so gathers don't stall
                        nxt = load_ibatch(q, bi + 1) if bi + 1 < nbatch else None
                        ibatch_tiles[q] = (cur, nxt)
                        ibatch_id[q] = bi
                    (it, st), _ = ibatch_tiles[q]
                    r = min(CH, int(TQ[q]) - ci * CH)
                    co = (ci % IB) * CH
                    gt = gp3.tile([128, CH, C], GD, tag=f"g{q}")
                    nc.gpsimd.dma_gather(
                        gt[:, :r, :], tab_ap(l, q),
                        it[:, co * 8:(co + r) * 8],
                        r * 128, r * 128, C, queue_num=0)
                    S = gpS.tile([128, CH, 128], GD, tag=f"S{q}", name=f"S{q}")
                    nc.vector.tensor_tensor(
                        out=S[:, :r, :],
                        in0=iota[:].unsqueeze(1).broadcast_to([128, r, 128]),
                        in1=st[:, co:co + r].unsqueeze(2)
                            .broadcast_to([128, r, 128]),
                        op=eq_.is_equal)
                    return (gt, S)

                def consume(b, qs, aggT_p):
                    ntot = int(sum(T[b, q] for q in qs))
                    done = 0
                    for q in qs:
                        nt = int(T[b, q])
                        t0 = cur[q]
                        cur[q] += nt
                        while nt > 0:
                            ci = t0 // CH
                            if chunk_id[q] != ci:
                                chunk_tiles[q] = issue_chunk(q, ci)
                                chunk_id[q] = ci
                            col = t0 % CH
                            r = min(nt, CH - col)
                            gt, S = chunk_tiles[q]
                            for j in range(r):
                                nc.tensor.matmul(
                                    aggT_p[:], gt[:, col + j, :], S[:, col + j, :],
                                    start=(done == 0), stop=(done == ntot - 1))
                                done += 1
                            t0 += r
                            nt -= r

                # ---------------- PASS A phase 1: buckets 0,1 ----------------
                scope1 = nc.named_scope(f"passA1_{l}"); scope1.__enter__()
                for b in range(NBLK):
                    aggT_p = ppA.tile([C, 128], f32, tag="aggT")
                    consume(b, (0, 1), aggT_p)
                    nc.scalar.activation(aggP[:, b, :], aggT_p[:], AF.Copy)
                scope1.__exit__(None, None, None)

                # ---------------- PASS A phase 2: buckets 2,3 ----------------
                # stats accumulate in halves; AR of the first half overlaps
                # the second half's gathers.
                scope2 = nc.named_scope(f"passA2_{l}"); scope2.__enter__()
                for b in range(NBLK):
                    aggT_p = ppA.tile([C, 128], f32, tag="aggT")
                    consume(b, (2, 3), aggT_p)
                    aggT_s = wp.tile([C, 128], f32, tag="aggTs")
                    nc.vector.tensor_tensor(out=aggT_s[:], in0=aggT_p[:],
                                            in1=aggP[:, b, :], op=eq_.add)
                    h_p = pp.tile([128, C], f32, tag="hp")
                    nc.tensor.matmul(h_p[:], aggT_s[:], wm[:, l, :],
                                     start=True, stop=True)
                    nc.scalar.activation(hbuf[:, b, :], h_p[:], AF.Copy,
                                         scale=invin[:, b:b + 1],
                                         accum_out=nm_arr[:, b:b + 1])
                    h2 = wp.tile([128, C], GD, tag="h2")
                    nc.scalar.activation(h2[:], hbuf[:, b, :], AF.Square,
                                         accum_out=nv_arr[:, b:b + 1])
                    nc.tensor.matmul(gs_p[:], goh16[:, b, :], hbuf[:, b, :],
                                     start=(b == 0 or b == HBLK),
                                     stop=(b == HBLK - 1 or b == NBLK - 1))
                    nc.tensor.matmul(gss_p[:], goh16[:, b, :], h2[:],
                                     start=(b == 0 or b == HBLK),
                                     stop=(b == HBLK - 1 or b == NBLK - 1))
                    if b == HBLK - 1:
                        stsA = kp.tile([B, 2 * C], f32, tag="stsA")
                        nc.vector.tensor_copy(stsA[:, 0:C], gs_p[:])
                        nc.vector.tensor_copy(stsA[:, C:2 * C], gss_p[:])
                        nc.sync.dma_start(stinA[l].ap(), stsA[:])
                        nc.gpsimd.collective_compute(
                            "AllReduce", eq_.add, ins=[stinA[l].ap()],
                            outs=[stoutA[l].ap()], replica_groups=RG)
                scope2.__exit__(None, None, None)

                scopeS = nc.named_scope(f"stats_{l}"); scopeS.__enter__()
                # ---- second-half stats AllReduce + combine ----
                sts = kp.tile([B, 2 * C], f32, tag="sts")
                nc.vector.tensor_copy(sts[:, 0:C], gs_p[:])
                nc.vector.tensor_copy(sts[:, C:2 * C], gss_p[:])
                nc.sync.dma_start(stin[l].ap(), sts[:])
                nc.gpsimd.collective_compute(
                    "AllReduce", eq_.add, ins=[stin[l].ap()],
                    outs=[stout[l].ap()], replica_groups=RG)
                gst = kp.tile([B, 2 * C], f32, tag="gst")
                nc.sync.dma_start(gst[:], stout[l].ap())
                gstA = kp.tile([B, 2 * C], f32, tag="gstA")
                nc.sync.dma_start(gstA[:], stoutA[l].ap())
                nc.vector.tensor_tensor(out=gst[:], in0=gst[:], in1=gstA[:],
                                        op=eq_.add)
                gs, gss = gst[:, 0:C], gst[:, C:2 * C]

                # ---- coefficients AB = [A16 | B16] (bf16) ----
                gm = kp.tile([B, C], f32, tag="gm")
                nc.vector.tensor_scalar_mul(gm[:], gs, cntinv[:])
                gv = kp.tile([B, C], f32, tag="gv")
                nc.vector.tensor_scalar_mul(gv[:], gss, cntinv[:])
                tmp16 = kp.tile([B, C], f32, tag="tmp16")
                nc.vector.tensor_tensor(out=tmp16[:], in0=gm[:], in1=gm[:], op=eq_.mult)
                nc.vector.tensor_tensor(out=gv[:], in0=gv[:], in1=tmp16[:], op=eq_.subtract)
                nc.scalar.activation(gv[:], gv[:], AF.Sqrt, bias=eps128[0:B, :])
                igv = kp.tile([B, C], f32, tag="igv")
                nc.vector.reciprocal(igv[:], gv[:])
                bs_p = ppA.tile([1, 2 * C], f32, tag="aggT")
                nc.tensor.matmul(bs_p[:], ones16[:], gst[:], start=True, stop=True)
                bm = kp.tile([1, C], f32, tag="bm")
                nc.vector.tensor_scalar_mul(bm[:], bs_p[:, 0:C], 1.0 / N)
                bv = kp.tile([1, C], f32, tag="bv")
                nc.vector.tensor_scalar_mul(bv[:], bs_p[:, C:2 * C], 1.0 / N)
                tmp1 = kp.tile([1, C], f32, tag="tmp1")
                nc.vector.tensor_tensor(out=tmp1[:], in0=bm[:], in1=bm[:], op=eq_.mult)
                nc.vector.tensor_tensor(out=bv[:], in0=bv[:], in1=tmp1[:], op=eq_.subtract)
                nc.scalar.activation(bv[:], bv[:], AF.Sqrt, bias=eps128[0:1, :])
                ibv = kp.tile([1, C], f32, tag="ibv")
                nc.vector.reciprocal(ibv[:], bv[:])
                # broadcast [1,C] rows to B partitions via K=1 matmul
                ibv_p = ppA.tile([B, C], f32, tag="aggT")
                nc.tensor.matmul(ibv_p[:], ones1[:], ibv[:], start=True, stop=True)
                bmibv = kp.tile([1, C], f32, tag="bmibv")
                nc.vector.tensor_tensor(out=bmibv[:], in0=bm[:], in1=ibv[:], op=eq_.mult)
                bmibv_p = pp.tile([B, C], f32, tag="hp")
                nc.tensor.matmul(bmibv_p[:], ones1[:], bmibv[:], start=True, stop=True)
                AB = kp.tile([B, 2 * C], f32, tag="AB")
                A16, B16 = AB[:, 0:C], AB[:, C:2 * C]
                nc.vector.tensor_scalar_mul(A16, igv[:], w1)
                t16b = kp.tile([B, C], f32, tag="t16b")
                nc.vector.tensor_scalar_mul(t16b[:], ibv_p[:], w0)
                nc.vector.tensor_tensor(out=A16, in0=A16, in1=t16b[:], op=eq_.add)
                nc.vector.tensor_tensor(out=B16, in0=gm[:], in1=igv[:], op=eq_.mult)
                nc.vector.tensor_scalar_mul(B16, B16, w1)
                nc.vector.tensor_scalar_mul(t16b[:], bmibv_p[:], w0)
                nc.vector.tensor_tensor(out=B16, in0=B16, in1=t16b[:], op=eq_.add)

                # ---- per-node coefficients: a_n = w2*invn, bn_neg = -nm*a_n ----
                nmm = kp.tile([128, NBLK], f32, tag="nmm")
                nc.vector.tensor_scalar_mul(nmm[:], nm_arr[:], 1.0 / C)
                nvm = kp.tile([128, NBLK], f32, tag="nvm")
                nc.vector.tensor_scalar_mul(nvm[:], nv_arr[:], 1.0 / C)
                nm2 = kp.tile([128, NBLK], f32, tag="nm2")
                nc.vector.tensor_tensor(out=nm2[:], in0=nmm[:], in1=nmm[:], op=eq_.mult)
                nc.vector.tensor_tensor(out=nvm[:], in0=nvm[:], in1=nm2[:], op=eq_.subtract)
                nc.scalar.activation(nvm[:], nvm[:], AF.Sqrt, bias=eps128[:])
                invn = kp.tile([128, NBLK], f32, tag="invn")
                nc.vector.reciprocal(invn[:], nvm[:])
                a_n = kp.tile([128, NBLK], f32, tag="a_n")
                nc.vector.tensor_scalar_mul(a_n[:], invn[:], w2)
                b_n = kp.tile([128, NBLK], f32, tag="b_n")
                nc.vector.tensor_tensor(out=b_n[:], in0=nmm[:], in1=a_n[:], op=eq_.mult)

                scopeS.__exit__(None, None, None)
                if l == L - 1:
                    emb_p = pp1.tile([B, C], f32, tag="emb")

                # ---------------- PASS B ----------------
                scopeB = nc.named_scope(f"passB_{l}"); scopeB.__enter__()
                for b in range(NBLK):
                    AB_p = ppA.tile([128, 2 * C], f32, tag="aggT")
                    nc.tensor.matmul(AB_p[:], gTall[:, b, :], AB[:],
                                     start=True, stop=True)
                    h = hbuf[:, b, :]
                    # u = h*(A_p + a_n) - (B_p + b_n): fold the per-node
                    # scalars into the per-graph tiles on ACT (PSUM reads).
                    Pt = wp.tile([128, C], f32, tag="Pt")
                    nc.scalar.activation(Pt[:], AB_p[:, 0:C], AF.Identity,
                                         bias=a_n[:, b:b + 1])
                    Qt = wp.tile([128, C], f32, tag="Qt")
                    nc.scalar.activation(Qt[:], AB_p[:, C:2 * C], AF.Identity,
                                         bias=b_n[:, b:b + 1])
                    v = wp.tile([128, C], f32, tag="v")
                    nc.vector.tensor_tensor(out=v[:], in0=h, in1=Pt[:], op=eq_.mult)
                    u = wp.tile([128, C], f32, tag="u")
                    nc.vector.tensor_tensor(out=u[:], in0=v[:], in1=Qt[:],
                                            op=eq_.subtract)
                    if not gtriv:
                        gam_p = pp.tile([128, C], f32, tag="hp")
                        nc.tensor.matmul(gam_p[:], ones1p[:], gam[l:l + 1, :],
                                         start=True, stop=True)
                        bet_p = pp.tile([128, C], f32, tag="hp2")
                        nc.tensor.matmul(bet_p[:], ones1p[:], bet[l:l + 1, :],
                                         start=True, stop=True)
                        nc.vector.tensor_tensor(out=u[:], in0=u[:], in1=gam_p[:], op=eq_.mult)
                        nc.vector.tensor_tensor(out=u[:], in0=u[:], in1=bet_p[:], op=eq_.add)
                    if l < L - 1:
                        # m = leaky(u)*invout = SLOPE*invout*u + (1-SLOPE)*relu(u*invout)
                        r_t = wp.tile([128, C], f32, tag="r_t")
                        nc.scalar.activation(r_t[:], u[:], AF.Relu,
                                             scale=invout08[:, b:b + 1])
                        nc.vector.tensor_scalar(u[:], u[:],
                                                invout02[:, b:b + 1], None,
                                                eq_.mult)
                        nc.vector.tensor_tensor(out=hbuf[:, b, :], in0=u[:],
                                                in1=r_t[:], op=eq_.add)
                        if b == HBLK - 1:
                            nc.sync.dma_start(
                                mshA[l].ap().rearrange("(b p) c -> p b c", p=128),
                                hbuf[:, 0:HBLK, :])
                            nc.gpsimd.collective_compute(
                                "AllGather", eq_.bypass, ins=[mshA[l].ap()],
                                outs=[mfA[l].ap()], replica_groups=RG)
                        elif b == NBLK - 1:
                            nc.sync.dma_start(
                                mshB[l].ap().rearrange("(b p) c -> p b c", p=128),
                                hbuf[:, HBLK:NBLK, :])
                            nc.gpsimd.collective_compute(
                                "AllGather", eq_.bypass, ins=[mshB[l].ap()],
                                outs=[mfB[l].ap()], replica_groups=RG)
                    else:
                        r_t = wp.tile([128, C], f32, tag="r_t")
                        nc.scalar.activation(r_t[:], u[:], AF.Relu,
                                             scale=1.0 - SLOPE)
                        nc.vector.tensor_scalar_mul(u[:], u[:], SLOPE)
                        lu = wp.tile([128, C], f32, tag="lu")
                        nc.vector.tensor_tensor(out=lu[:], in0=u[:],
                                                in1=r_t[:], op=eq_.add)
                        nc.tensor.matmul(emb_p[:], goh[:, b, :], lu[:],
                                         start=(b == 0), stop=(b == NBLK - 1))
                scopeB.__exit__(None, None, None)

            # ---- readout ----
            embs = kp.tile([B, C], f32, tag="embs")
            nc.vector.tensor_copy(embs[:], emb_p[:])
            nc.sync.dma_start(embin.ap(), embs[:])
            nc.gpsimd.collective_compute(
                "AllReduce", eq_.add, ins=[embin.ap()], outs=[embout.ap()],
                replica_groups=RG)
            embg = kp.tile([B, C], f32, tag="embg")
            nc.sync.dma_start(embg[:], embout.ap())
            nc.vector.tensor_scalar_mul(embg[:], embg[:], cntinv[:])
            embg2 = kp.tile([B, C], f32, tag="embg2")
            nc.vector.tensor_scalar_mul(embg2[:], embg[:], SLOPE)
            nc.vector.tensor_tensor(out=embg[:], in0=embg[:], in1=embg2[:], op=eq_.max)
            nc.sync.dma_start(out_t.ap(), embg[:])

    nc.compile()
    _fixup_swdge_queues(nc)
    return nc


def _fixup_swdge_queues(nc):
    """Rewrite gather queue_num in scheduled (module) order.

    Tile assigns DMASW completion-sem lanes round-robin (mod 8) over
    Pool-engine DMA instructions in module order, and the ucode requires a
    sem lane to be incremented from a single SWDGE queue. Setting
    queue = lane % 4 in the same walk order satisfies that for any schedule
    while keeping all 4 queues busy.
    """
    import concourse.mybir as mybir
    pool_dma = ("InstDMAGatherAnt", "InstDMAScatterAddAnt",
                "InstKVWritebackAnt", "InstPagedWritebackAnt")
    lane = 0
    for f in nc.m.functions:
        for bb in f.blocks:
            for inst in bb.instructions:
                cls = type(inst).__name__
                is_pool_dma = cls in pool_dma or (
                    cls in ("InstDMACopy", "InstTensorLoad", "InstTensorSave")
                    and getattr(inst, "engine", None) == mybir.EngineType.Pool)
                if is_pool_dma:
                    inst.queue_num = lane % 4
                    lane += 1


def make_in_maps(cfg, per_core, consts):
    import ml_dtypes
    GD_np = np.float32 if cfg["GD"] == "f32" else ml_dtypes.bfloat16
    base = dict(
        m1tab=consts["m1tab"].astype(GD_np),
        iota=consts["iota"].astype(GD_np), wmat=consts["wmat"],
        cnt_inv=consts["cnt_inv"], gamma=consts["gamma"], beta=consts["beta"],
    )
    in_maps = []
    for k in range(cfg["NCORES"]):
        d = dict(base)
        for kk, vv in per_core[k].items():
            if kk.startswith("slotq"):
                vv = vv.astype(GD_np)
            d[kk] = vv
        in_maps.append(d)
    return in_maps


_BUILD_CACHE = {}


def _build_cached(cfg, meta):
    key = (tuple(sorted((k, str(v)) for k, v in cfg.items())),
           meta["T"].tobytes(), meta["wsoft"].tobytes(), meta["gamma_trivial"])
    if key not in _BUILD_CACHE:
        _BUILD_CACHE[key] = build_nc(cfg, meta)
    return _BUILD_CACHE[key]


def run_cfg(cfg, inputs, trace=False, verbose=False):
    import time
    from concourse.bass_utils import run_bass_kernel_spmd
    t0 = time.time()
    cfg = _derive(cfg)
    meta, per_core, consts = prep_host(inputs, cfg)
    t1 = time.time()
    nc = _build_cached(cfg, meta)
    t2 = time.time()
    in_maps = make_in_maps(cfg, per_core, consts)
    res = run_bass_kernel_spmd(nc, in_maps, list(range(cfg["NCORES"])),
                               trace=trace)
    t3 = time.time()
    if verbose:
        print(f"[timing] prep={t1-t0:.1f}s build+compile={t2-t1:.1f}s "
              f"run={t3-t2:.1f}s", flush=True)
    return res.results[0]["out"].astype(np.float32), res


def kernel(**inputs):
    out, _ = run_cfg(_cfg_real(), inputs)
    return out
